# revision 1
# baseline (speedup 1.0000x reference)
"""GroupedQueryAttention Trainium2 kernel (8 NeuronCores).

Sharding: core c -> (kv-group g = c%4, head-slot pair {2*(c//4), 2*(c//4)+1}).
Each core computes its group's two head-slots over BOTH batches (K/V are
computed per batch on every core), then two 8-rank AllToAlls (one per
head-slot j) redistribute ctx^T so core c ends up with ctx^T of ALL 16
heads for its own flat row chunk c (batch c//4, rows 512*(c%4)..+512).
The output projection is then row-parallel with the full Wo resident —
no further collectives.

Every AllToAll block is useful: src s = (g=s%4, half=s//4) contributes
head (g, 2*half+j) of batch d//4, rows 512*(d%4).. to dest d, and the
d_model chunk index of block (s, j) is the compile-time constant
4*(s%4) + 2*(s//4) + j — pure SPMD, no rank-dependent indexing.

Pipelining: the j=0 attention units are emitted interleaved with the
projection chunks so their exp runs on the otherwise-idle Activation
engine during phase 1, letting the first AllToAll issue right after the
projections drain. The output projection runs in two waves: bias + j=0
terms accumulate into an SBUF fp32 accumulator while the second
AllToAll is still in flight; the j=1 terms and the final add/store run
after it lands.

Layout trick: scores are computed transposed (S^T[k, q]) so A^T =
exp(S^T) is directly the lhsT of the ctx matmul. The softmax denominator
comes free as a 129th "ones" column appended to V; ctx rows are
normalized by a per-partition reciprocal scale. Score blocks are packed
into 512-col PSUM bank slots with fully-masked diagonal columns trimmed,
so exp reads one contiguous range per [128,1024] tile.
"""

from contextlib import ExitStack

import numpy as np
import ml_dtypes

import concourse.bass as bass
import concourse.bacc as bacc
import concourse.tile as tile
from concourse import mybir
from concourse.bass_utils import run_bass_kernel_spmd
from concourse.masks import make_identity
from concourse.tile_rust import add_dep_helper

BF16 = mybir.dt.bfloat16
F32 = mybir.dt.float32

B = 2
S = 2048
D = 2048
G = 4  # kv groups
HPG = 4  # heads per group
HD = 128  # head dim
QC = 512  # q-chunk (columns per S^T block)
NQC = S // QC  # 4
NKT = S // 128  # 16 k-tiles
NDC = D // 128  # 16 d_in chunks
SCALE = 1.0 / float(np.sqrt(HD))
N_CORES = 8
REPLICA_GROUPS = [[0, 1, 2, 3, 4, 5, 6, 7]]

CP = mybir.ActivationFunctionType.Copy
EXP = mybir.ActivationFunctionType.Exp


def _build_program():
    nc = bacc.Bacc("TRN2", target_bir_lowering=False, debug=False)

    xq = nc.declare_dram_parameter("xq", [B, NQC, NDC, 128, QC], BF16, isOutput=False)
    wq = nc.declare_dram_parameter("wq", [NDC, 128, 2 * HD], BF16, isOutput=False)
    wk = nc.declare_dram_parameter("wk", [NDC, 128, HD], BF16, isOutput=False)
    wv = nc.declare_dram_parameter("wv", [NDC, 128, HD], BF16, isOutput=False)
    wo = nc.declare_dram_parameter("wo", [NDC, 128, D], BF16, isOutput=False)
    bo = nc.declare_dram_parameter("bo", [1, D], BF16, isOutput=False)
    msk = nc.declare_dram_parameter("msk", [128, 128], BF16, isOutput=False)
    out_ext = nc.declare_dram_parameter("out", [QC, D], F32, isOutput=True)

    # AllToAll outputs: recv[j] block s = ctx^T of (g=s%4, h=2*(s//4)+j)
    # for this core's flat row chunk
    recv0 = nc.dram_tensor("recv0", [N_CORES, HD, QC], BF16)
    recv1 = nc.dram_tensor("recv1", [N_CORES, HD, QC], BF16)

    with tile.TileContext(nc) as tc, ExitStack() as es:
        singles = es.enter_context(tc.tile_pool(name="singles", bufs=1))
        wpool = es.enter_context(tc.tile_pool(name="w", bufs=1))
        qkpool = es.enter_context(tc.tile_pool(name="qk", bufs=1))
        apool = es.enter_context(tc.tile_pool(name="a", bufs=12))
        spool = es.enter_context(tc.tile_pool(name="sm", bufs=1))
        cpool = es.enter_context(tc.tile_pool(name="cs", bufs=6))
        opool = es.enter_context(tc.tile_pool(name="ob", bufs=2))
        psc = es.enter_context(tc.tile_pool(name="psc", bufs=2, space="PSUM"))
        pss = es.enter_context(tc.tile_pool(name="pss", bufs=2, space="PSUM"))
        pst = es.enter_context(tc.tile_pool(name="pst", bufs=2, space="PSUM"))
        dram = es.enter_context(tc.tile_pool(name="dram", bufs=1, space="DRAM"))
        dram2 = es.enter_context(tc.tile_pool(name="dram2", bufs=1, space="DRAM"))

        # --- constants (off the SP queue so x strip 0 starts immediately) ---
        ident = singles.tile([128, 128], BF16, tag="ident")
        make_identity(nc, ident)
        ones1 = singles.tile([1, 128], BF16, tag="ones1")
        nc.vector.memset(ones1, 1.0)
        bo_sb = singles.tile([1, D], BF16, tag="bo")
        nc.scalar.dma_start(out=bo_sb, in_=bo[:, :])
        mask_sb = singles.tile([128, 128], BF16, tag="mask")
        nc.scalar.dma_start(out=mask_sb, in_=msk[:, :])
        # preload the exp activation table so phase 2's first exp is cheap
        warm = singles.tile([1, 4], F32, tag="warm")
        nc.scalar.activation(out=warm, in_=ones1[:, 0:4], func=EXP)

        # --- resident weights, loaded on the gpsimd queue (idle early) ---
        wqall = wpool.tile([128, NDC, 2 * HD], BF16, tag="wqall")
        nc.gpsimd.dma_start(out=wqall, in_=wq.rearrange("a p q -> p a q"))
        wkall = wpool.tile([128, NDC, HD], BF16, tag="wkall")
        nc.gpsimd.dma_start(out=wkall, in_=wk.rearrange("a p q -> p a q"))
        wvall = wpool.tile([128, NDC, HD], BF16, tag="wvall")
        nc.gpsimd.dma_start(out=wvall, in_=wv.rearrange("a p q -> p a q"))
        woall = wpool.tile([128, NDC, D], BF16, tag="woall")
        nc.gpsimd.dma_start(out=woall, in_=wo.rearrange("a p q -> p a q"))
        wq_sb = [wqall[:, dc, :] for dc in range(NDC)]
        wk_sb = [wkall[:, dc, :] for dc in range(NDC)]
        wv_sb = [wvall[:, dc, :] for dc in range(NDC)]

        # --- persistent activations (per batch) ---
        qT = [
            [
                qkpool.tile([128, S], BF16, tag=f"qT{b}{j}", name=f"qT{b}{j}")
                for j in range(2)
            ]
            for b in range(B)
        ]
        kT = [
            qkpool.tile([128, S], BF16, tag=f"kT{b}", name=f"kT{b}") for b in range(B)
        ]
        vext = [
            [
                qkpool.tile([128, HD + 1], BF16, tag=f"v{b}_{i}", name=f"v{b}_{i}")
                for i in range(NKT)
            ]
            for b in range(B)
        ]

        ct_dram0 = dram.tile([N_CORES, HD, QC], BF16, tag="ct0", name="ctd0")
        ct_dram1 = dram2.tile([N_CORES, HD, QC], BF16, tag="ct1", name="ctd1")

        def attn_unit(b, j, qc):
            """Attention for (batch b, head-slot j, q-chunk qc); writes
            ctx^T [128, 512] to ct_dram[j] block 4*b+qc."""
            d0 = 4 * qc
            # 512-col PSUM bank slots; diagonal tiles trimmed to live cols
            slots = [[(kt, 0, QC)] for kt in range(d0)]
            slots.append([(d0, 0, QC)])
            slots.append([(d0 + 1, 128, 384), (d0 + 3, 384, 128)])
            slots.append([(d0 + 2, 256, 256)])  # partial slot last
            groups = [slots[p : p + 2] for p in range(0, len(slots), 2)]
            a_sl = {}

            def emit_group(grp):
                ps = psc.tile([128, 2 * QC], F32, tag="sc")
                a = apool.tile([128, 2 * QC], BF16, tag="a")
                width = 0
                for si, slot in enumerate(grp):
                    soff = si * QC
                    for kt, qb, w in slot:
                        nc.tensor.matmul(
                            ps[:, soff : soff + w],
                            lhsT=kT[b][:, kt * 128 : (kt + 1) * 128],
                            rhs=qT[b][j][:, qc * QC + qb : (qc + 1) * QC],
                            start=True,
                            stop=True,
                        )
                        a_sl[kt] = (a, soff, qb)
                        soff += w
                    width = soff
                nc.scalar.activation(
                    out=a[:, 0:width], in_=ps[:, 0:width], func=EXP, scale=SCALE
                )
                for slot in grp:
                    for kt, qb, w in slot:
                        if kt >= d0:  # diagonal: triangular mask
                            ao = a_sl[kt][1]
                            nc.vector.tensor_mul(
                                a[:, ao : ao + 128], a[:, ao : ao + 128], mask_sb
                            )

            ct = cpool.tile([128, QC], BF16, tag="ct", bufs=12)

            def emit_ctx(st):
                qt = qc * 4 + st
                cps = pss.tile([128, HD + 1], F32, tag="small")
                for kt in range(qt + 1):
                    a, ao, qb = a_sl[kt]
                    nc.tensor.matmul(
                        cps,
                        lhsT=a[:, ao + st * 128 - qb : ao + (st + 1) * 128 - qb],
                        rhs=vext[b][kt],
                        start=(kt == 0),
                        stop=(kt == qt),
                    )
                zr = cpool.tile([128, 1], F32, tag="zr", bufs=6)
                nc.vector.reciprocal(zr, cps[:, HD : HD + 1])
                cs = cpool.tile([128, HD], BF16, tag="cs", bufs=6)
                nc.vector.tensor_scalar_mul(cs, cps[:, 0:HD], zr)
                tp = pst.tile([128, 128], BF16, tag="tp")
                nc.tensor.transpose(tp, cs, ident)
                nc.vector.tensor_copy(ct[:, st * 128 : (st + 1) * 128], tp)

            # groups 0..2qc cover k-tiles d0, d0+1, d0+3 -> st 0,1,3 ready;
            # the final group only adds d0+2 (needed by st 2,3)
            for grp in groups[:-1]:
                emit_group(grp)
            emit_ctx(0)
            emit_ctx(1)
            emit_group(groups[-1])
            emit_ctx(2)
            emit_ctx(3)
            if j == 0:
                return nc.sync.dma_start(out=ct_dram0[4 * b + qc], in_=ct)
            return nc.sync.dma_start(out=ct_dram1[4 * b + qc], in_=ct)

        # ===== Phase 1 + interleaved j=0 attention (one chunk late, so each
        # unit's exp has a full projection chunk of slack before its ctx
        # matmuls reach the head of the in-order PE queue) =====
        with tc.tile_pool(name="x", bufs=2) as xpool:
            pending = []  # deferred j=0 attention units

            def proj_chunk(b, qc):
                xstrip = xpool.tile([128, NDC, QC], BF16, tag="xs")
                if b == 0 and qc == 0:
                    for dq in range(4):
                        nc.sync.dma_start(
                            out=xstrip[:, 4 * dq : 4 * (dq + 1), :],
                            in_=xq[b, qc, 4 * dq : 4 * (dq + 1)].rearrange(
                                "a p q -> p a q"
                            ),
                        )
                else:
                    nc.sync.dma_start(
                        out=xstrip, in_=xq[b, qc].rearrange("a p q -> p a q")
                    )
                xs = [xstrip[:, dc, :] for dc in range(NDC)]
                # Q^T both head-slots packed into one [128,1024] PSUM tile
                psq = psc.tile([128, 2 * QC], F32, tag="sc")
                for j in range(2):
                    for dc in range(NDC):
                        nc.tensor.matmul(
                            psq[:, j * QC : (j + 1) * QC],
                            lhsT=wq_sb[dc][:, j * HD : (j + 1) * HD],
                            rhs=xs[dc],
                            start=(dc == 0),
                            stop=(dc == NDC - 1),
                        )
                    nc.scalar.activation(
                        out=qT[b][j][:, qc * QC : (qc + 1) * QC],
                        in_=psq[:, j * QC : (j + 1) * QC],
                        func=CP,
                    )
                # K^T: [dh, q 512]
                psk = psc.tile([128, 2 * QC], F32, tag="sc")
                for dc in range(NDC):
                    nc.tensor.matmul(
                        psk[:, 0:QC],
                        lhsT=wk_sb[dc],
                        rhs=xs[dc],
                        start=(dc == 0),
                        stop=(dc == NDC - 1),
                    )
                nc.scalar.activation(
                    out=kT[b][:, qc * QC : (qc + 1) * QC],
                    in_=psk[:, 0:QC],
                    func=CP,
                )
                # V: [s-tile 128, dv 128] (natural orientation)
                for st in range(4):
                    kt = qc * 4 + st
                    ps = pss.tile([128, HD + 1], F32, tag="small")
                    for dc in range(NDC):
                        nc.tensor.matmul(
                            ps[:, 0:HD],
                            lhsT=xs[dc][:, st * 128 : (st + 1) * 128],
                            rhs=wv_sb[dc],
                            start=(dc == 0),
                            stop=(dc == NDC - 1),
                        )
                    nc.scalar.activation(
                        out=vext[b][kt][:, 0:HD], in_=ps[:, 0:HD], func=CP
                    )
                    nc.vector.memset(vext[b][kt][:, HD : HD + 1], 1.0)

            for b in range(B):
                for qc in range(NQC):
                    proj_chunk(b, qc)
                    pending.append((b, qc))
                    if len(pending) > 1:
                        pb, pqc = pending.pop(0)
                        attn_unit(pb, 0, pqc)
            for pb, pqc in pending:
                attn_unit(pb, 0, pqc)

        coll0 = nc.gpsimd.collective_compute(
            "AllToAll",
            mybir.AluOpType.bypass,
            replica_groups=REPLICA_GROUPS,
            ins=[ct_dram0[:, :, :].opt()],
            outs=[recv0[:, :, :].opt()],
        )

        # ===== j=1 attention (descending qc so the longest unit isn't the
        # second collective's latency tail) =====
        last_ct = None
        for b in range(B):
            for qc in reversed(range(NQC)):
                last_ct = attn_unit(b, 1, qc)
        coll1 = nc.gpsimd.collective_compute(
            "AllToAll",
            mybir.AluOpType.bypass,
            replica_groups=REPLICA_GROUPS,
            ins=[ct_dram1[:, :, :].opt()],
            outs=[recv1[:, :, :].opt()],
        )
        colls = [coll0, coll1]

        # ===== Phase 3: row-parallel output projection, two waves =====
        accp = es.enter_context(tc.tile_pool(name="acc", bufs=1))
        accs = [
            accp.tile([128, D], F32, tag=f"acc{st}", name=f"acc{st}")
            for st in range(4)
        ]
        cstrips = []
        cstrip0 = spool.tile([128, N_CORES, QC], BF16, tag="cstrip0")
        d = nc.sync.dma_start(
            out=cstrip0, in_=recv0[:, :, :].rearrange("g p q -> p g q")
        )
        add_dep_helper(d.ins, coll0.ins, reason="alltoall->read")
        # keep the scheduler from hoisting this read ahead of the j=1 ct
        # writes on the SP queue (it would chain them behind coll0)
        add_dep_helper(d.ins, last_ct.ins, reason="order-after-ct-writes")
        cstrips.append(cstrip0)
        cstrip1 = spool.tile([128, N_CORES, QC], BF16, tag="cstrip1")
        d = nc.sync.dma_start(
            out=cstrip1, in_=recv1[:, :, :].rearrange("g p q -> p g q")
        )
        add_dep_helper(d.ins, coll1.ins, reason="alltoall->read")
        add_dep_helper(d.ins, last_ct.ins, reason="order-after-ct-writes")
        cstrips.append(cstrip1)

        units = [(st, cc) for st in range(4) for cc in range(4)]
        unitsB = [(st, cc) for st in (0, 1, 2, 3) for cc in range(4)]

        def wave(j, final):
            ulist = unitsB if final else units
            for u in range(0, len(ulist), 2):
                psu = psc.tile([128, 2 * QC], F32, tag="sc")
                for k, (st, cc) in enumerate(ulist[u : u + 2]):
                    half = psu[:, k * QC : (k + 1) * QC]
                    if not final:
                        nc.tensor.matmul(
                            half,
                            lhsT=ones1,
                            rhs=bo_sb[:, cc * QC : (cc + 1) * QC],
                            start=True,
                            stop=False,
                        )
                    for s in range(N_CORES):
                        dch = 4 * (s % 4) + 2 * (s // 4) + j
                        nc.tensor.matmul(
                            half,
                            lhsT=cstrips[j][:, s, st * 128 : (st + 1) * 128],
                            rhs=woall[:, dch, cc * QC : (cc + 1) * QC],
                            start=(final and s == 0),
                            stop=(s == N_CORES - 1),
                        )
                    if not final:
                        if (u + k) % 2 == 0:
                            nc.vector.tensor_copy(
                                accs[st][:, cc * QC : (cc + 1) * QC], half
                            )
                        else:
                            nc.scalar.activation(
                                out=accs[st][:, cc * QC : (cc + 1) * QC],
                                in_=half,
                                func=CP,
                            )
                    else:
                        osb = opool.tile([128, QC], F32, tag="osb")
                        nc.vector.tensor_add(
                            osb, half, accs[st][:, cc * QC : (cc + 1) * QC]
                        )
                        nc.sync.dma_start(
                            out=out_ext[
                                st * 128 : (st + 1) * 128, cc * QC : (cc + 1) * QC
                            ],
                            in_=osb,
                        )

        wave(0, final=False)
        wave(1, final=True)

    nc.compile()
    return nc


def _make_mask() -> np.ndarray:
    # mask[k, q] = 1.0 if q >= k (triangular causal for the [128,128]
    # diagonal sub-block of each diagonal k-tile)
    q = np.arange(128)[None, :]
    k = np.arange(128)[:, None]
    return (q >= k).astype(ml_dtypes.bfloat16)


def _make_in_maps(inputs) -> list[dict]:
    x = np.asarray(inputs["x"], dtype=np.float32)
    Wq = np.asarray(inputs["Wq"], dtype=np.float32)
    Wk = np.asarray(inputs["Wk"], dtype=np.float32)
    Wv = np.asarray(inputs["Wv"], dtype=np.float32)
    Wo = np.asarray(inputs["Wo"], dtype=np.float32)
    bo = np.asarray(inputs["bo"], dtype=np.float32)

    bf = ml_dtypes.bfloat16
    mask = _make_mask()

    # x^T tiled: [b, qc, dc, 128, 512], both batches shipped to every core
    xqs = []
    for b in range(B):
        xT = np.ascontiguousarray(x[b].T.astype(bf))  # [d, s]
        xqs.append(xT.reshape(NDC, 128, NQC, QC).transpose(2, 0, 1, 3))
    xq_all = np.ascontiguousarray(np.stack(xqs))

    wo_full = np.ascontiguousarray(Wo.astype(bf).reshape(NDC, 128, D))
    bo_full = np.ascontiguousarray(bo.astype(bf).reshape(1, D))

    in_maps = []
    for c in range(N_CORES):
        g, half = c % 4, c // 4
        q_lo = g * 512 + half * 2 * HD  # this core's two head-slots of group g
        in_maps.append(
            {
                "xq": xq_all,
                "wq": np.ascontiguousarray(
                    Wq[:, q_lo : q_lo + 2 * HD].astype(bf).reshape(NDC, 128, 2 * HD)
                ),
                "wk": np.ascontiguousarray(
                    Wk[:, g * HD : (g + 1) * HD].astype(bf).reshape(NDC, 128, HD)
                ),
                "wv": np.ascontiguousarray(
                    Wv[:, g * HD : (g + 1) * HD].astype(bf).reshape(NDC, 128, HD)
                ),
                "wo": wo_full,
                "bo": bo_full,
                "msk": mask,
            }
        )
    return in_maps


def _assemble(results) -> np.ndarray:
    out = np.empty((B, S, D), dtype=np.float32)
    for c in range(N_CORES):
        b, r = c // 4, c % 4
        out[b][r * QC : (r + 1) * QC, :] = results[c]["out"]
    return out


def kernel(**inputs) -> np.ndarray:
    in_maps = _make_in_maps(inputs)
    nc = _build_program()
    res = run_bass_kernel_spmd(nc, in_maps, list(range(N_CORES)))
    return _assemble(res.results)



# revision 27
# speedup vs baseline: 1.0787x; 1.0787x over previous
"""GroupedQueryAttention Trainium2 kernel (8 NeuronCores).

Sharding: core c -> (kv-group g = c%4, head-slot pair {2*(c//4), 2*(c//4)+1}).
Each core computes its group's two head-slots over BOTH batches (K/V are
computed per batch on every core), then two 8-rank AllToAlls (one per
head-slot j) redistribute ctx^T so core c ends up with ctx^T of ALL 16
heads for its own flat row chunk c (batch c//4, rows 512*(c%4)..+512).
The output projection is then row-parallel with the full Wo resident —
no further collectives.

Every AllToAll block is useful: src s = (g=s%4, half=s//4) contributes
head (g, 2*half+j) of batch d//4, rows 512*(d%4).. to dest d, and the
d_model chunk index of block (s, j) is the compile-time constant
4*(s%4) + 2*(s//4) + j — pure SPMD, no rank-dependent indexing.

Pipelining: the j=0 attention units are emitted interleaved with the
projection chunks so their exp runs on the otherwise-idle Activation
engine during phase 1, letting the first AllToAll issue right after the
projections drain. The output projection runs in two waves: bias + j=0
terms accumulate into an SBUF fp32 accumulator while the second
AllToAll is still in flight; the j=1 terms and the final add/store run
after it lands.

Layout trick: scores are computed transposed (S^T[k, q]) so A^T =
exp(S^T) is directly the lhsT of the ctx matmul. The softmax denominator
comes free as a 129th "ones" column appended to V; ctx rows are
normalized by a per-partition reciprocal scale. Score blocks are packed
into 512-col PSUM bank slots with fully-masked diagonal columns trimmed,
so exp reads one contiguous range per [128,1024] tile.
"""

from contextlib import ExitStack

import numpy as np
import ml_dtypes

import concourse.bass as bass
import concourse.bacc as bacc
import concourse.tile as tile
from concourse import mybir
from concourse.bass_utils import run_bass_kernel_spmd
from concourse.masks import make_identity
from concourse.tile_rust import add_dep_helper

BF16 = mybir.dt.bfloat16
F32 = mybir.dt.float32

B = 2
S = 2048
D = 2048
G = 4  # kv groups
HPG = 4  # heads per group
HD = 128  # head dim
QC = 512  # q-chunk (columns per S^T block)
NQC = S // QC  # 4
NKT = S // 128  # 16 k-tiles
NDC = D // 128  # 16 d_in chunks
SCALE = 1.0 / float(np.sqrt(HD))
N_CORES = 8
REPLICA_GROUPS = [[0, 1, 2, 3, 4, 5, 6, 7]]

CP = mybir.ActivationFunctionType.Copy
EXP = mybir.ActivationFunctionType.Exp


def _build_program(repeat: int = 1, sim: bool = False):
    nc = bacc.Bacc("TRN2", target_bir_lowering=False, debug=False)

    xq = nc.declare_dram_parameter("xq", [B, NQC, NDC, 128, QC], BF16, isOutput=False)
    wq = nc.declare_dram_parameter("wq", [NDC, 128, 2 * HD], BF16, isOutput=False)
    wk = nc.declare_dram_parameter("wk", [NDC, 128, HD], BF16, isOutput=False)
    wv = nc.declare_dram_parameter("wv", [NDC, 128, HD], BF16, isOutput=False)
    wo = nc.declare_dram_parameter("wo", [NDC, 128, D], BF16, isOutput=False)
    bo = nc.declare_dram_parameter("bo", [1, D], BF16, isOutput=False)
    msk = nc.declare_dram_parameter("msk", [128, 128], BF16, isOutput=False)
    out_ext = nc.declare_dram_parameter("out", [QC, D], BF16, isOutput=True)

    # AllToAll outputs: recv[j] block s = ctx^T of (g=s%4, h=2*(s//4)+j)
    # for this core's flat row chunk
    recv0 = nc.dram_tensor("recv0", [N_CORES, HD, QC], BF16)
    recv1 = nc.dram_tensor("recv1", [N_CORES, HD, QC], BF16)

    for _rep in range(repeat):
        _build_body(nc, _rep, xq, wq, wk, wv, wo, bo, msk, out_ext, recv0, recv1,
                    sim=sim)

    nc.compile()
    return nc


def _build_sim_program():
    """Single-core, collective-free variant of the body for TimelineSim."""
    nc = bacc.Bacc("TRN2", target_bir_lowering=False, debug=False)
    xq = nc.declare_dram_parameter("xq", [B, NQC, NDC, 128, QC], BF16, isOutput=False)
    wq = nc.declare_dram_parameter("wq", [NDC, 128, 2 * HD], BF16, isOutput=False)
    wk = nc.declare_dram_parameter("wk", [NDC, 128, HD], BF16, isOutput=False)
    wv = nc.declare_dram_parameter("wv", [NDC, 128, HD], BF16, isOutput=False)
    wo = nc.declare_dram_parameter("wo", [NDC, 128, D], BF16, isOutput=False)
    bo = nc.declare_dram_parameter("bo", [1, D], BF16, isOutput=False)
    msk = nc.declare_dram_parameter("msk", [128, 128], BF16, isOutput=False)
    out_ext = nc.declare_dram_parameter("out", [QC, D], BF16, isOutput=True)
    recv0 = nc.dram_tensor("recv0", [N_CORES, HD, QC], BF16)
    recv1 = nc.dram_tensor("recv1", [N_CORES, HD, QC], BF16)
    _build_body(nc, 0, xq, wq, wk, wv, wo, bo, msk, out_ext, recv0, recv1, sim=True)
    nc.compile()
    return nc


def _build_body(nc, _rep, xq, wq, wk, wv, wo, bo, msk, out_ext, recv0, recv1,
                sim=False):
    with tile.TileContext(nc) as tc, ExitStack() as es:
        singles = es.enter_context(tc.tile_pool(name="singles", bufs=1))
        wpool = es.enter_context(tc.tile_pool(name="w", bufs=1))
        qkpool = es.enter_context(tc.tile_pool(name="qk", bufs=1))
        apool = es.enter_context(tc.tile_pool(name="a", bufs=24))
        spool = es.enter_context(tc.tile_pool(name="sm", bufs=1))
        cpool = es.enter_context(tc.tile_pool(name="cs", bufs=6))
        opool = es.enter_context(tc.tile_pool(name="ob", bufs=2))
        pssc = es.enter_context(tc.tile_pool(name="pssc", bufs=4, space="PSUM"))
        pss = es.enter_context(tc.tile_pool(name="pss", bufs=2, space="PSUM"))
        pst = es.enter_context(tc.tile_pool(name="pst", bufs=2, space="PSUM"))
        dram = es.enter_context(tc.tile_pool(name="dram", bufs=1, space="DRAM"))
        dram2 = es.enter_context(tc.tile_pool(name="dram2", bufs=1, space="DRAM"))

        # --- constants (off the SP queue so x strip 0 starts immediately) ---
        ident = singles.tile([128, 128], BF16, tag="ident")
        make_identity(nc, ident)
        ones1 = singles.tile([1, 128], BF16, tag="ones1")
        nc.vector.memset(ones1, 1.0)
        bo_sb = singles.tile([1, D], BF16, tag="bo")
        nc.scalar.dma_start(out=bo_sb, in_=bo[:, :])
        mask_sb = singles.tile([128, 128], BF16, tag="mask")
        nc.scalar.dma_start(out=mask_sb, in_=msk[:, :])
        # preload the exp activation table so phase 2's first exp is cheap
        warm = singles.tile([1, 4], F32, tag="warm")
        nc.scalar.activation(out=warm, in_=ones1[:, 0:4], func=EXP)

        # --- resident weights, loaded on the gpsimd queue (idle early).
        # wq is split so the first Q matmuls (dc 0..3) start sooner; wo
        # (8.4 MB, ~23 us of DMA) is deferred until the x strips are ahead
        # of the PE (it is only needed in phase 3) to avoid starving the
        # strip loads mid-phase-1.
        wqall = wpool.tile([128, NDC, 2 * HD], BF16, tag="wqall")
        nc.gpsimd.dma_start(
            out=wqall[:, 0:4, :], in_=wq[0:4].rearrange("a p q -> p a q")
        )
        nc.gpsimd.dma_start(
            out=wqall[:, 4:NDC, :], in_=wq[4:NDC].rearrange("a p q -> p a q")
        )
        wkall = wpool.tile([128, NDC, HD], BF16, tag="wkall")
        nc.gpsimd.dma_start(out=wkall, in_=wk.rearrange("a p q -> p a q"))
        wvall = wpool.tile([128, NDC, HD], BF16, tag="wvall")
        nc.gpsimd.dma_start(out=wvall, in_=wv.rearrange("a p q -> p a q"))
        woall = wpool.tile([128, NDC, D], BF16, tag="woall")
        wq_sb = [wqall[:, dc, :] for dc in range(NDC)]
        wk_sb = [wkall[:, dc, :] for dc in range(NDC)]
        wv_sb = [wvall[:, dc, :] for dc in range(NDC)]

        # --- persistent activations (per batch) ---
        qT = [
            [
                qkpool.tile([128, S], BF16, tag=f"qT{b}{j}", name=f"r{_rep}qT{b}{j}")
                for j in range(2)
            ]
            for b in range(B)
        ]
        kT = [
            qkpool.tile([128, S], BF16, tag=f"kT{b}", name=f"r{_rep}kT{b}") for b in range(B)
        ]
        vext = [
            [
                qkpool.tile([128, HD + 1], BF16, tag=f"v{b}_{i}", name=f"r{_rep}v{b}_{i}")
                for i in range(NKT)
            ]
            for b in range(B)
        ]

        ct_dram0 = dram.tile([N_CORES, HD, QC], BF16, tag="ct0", name=f"r{_rep}ctd0")
        ct_dram1 = dram2.tile([N_CORES, HD, QC], BF16, tag="ct1", name=f"r{_rep}ctd1")

        # Deferred-finish queue: the transpose (PE) + copy (DVE) tail of each
        # ctx tile is emitted LAG ctx-tiles late, so by the time the PE
        # in-order queue reaches a transpose its DVE-normalized input is
        # already done and the PE never stalls on the DVE chain.
        fin_q = []
        FIN_LAG = 2
        last_ct_ref = [None]

        def push_fin(f):
            fin_q.append(f)
            while len(fin_q) > FIN_LAG:
                fin_q.pop(0)()

        def flush_fins():
            while fin_q:
                fin_q.pop(0)()

        def attn_unit(b, j, qc):
            """Attention for (batch b, head-slot j, q-chunk qc); writes
            ctx^T [128, 512] to ct_dram[j] block 4*b+qc."""
            d0 = 4 * qc
            # 512-col PSUM bank slots; diagonal tiles trimmed to live cols.
            # Each slot gets its own 1-bank PSUM tile + exp, so a score
            # matmul only WAR-waits on the exp from 4 slots ago (not the
            # 1024-wide exp of the previous slot pair).
            slots = [[(kt, 0, QC)] for kt in range(d0)]
            slots.append([(d0, 0, QC)])
            slots.append([(d0 + 1, 128, 384), (d0 + 3, 384, 128)])
            slots.append([(d0 + 2, 256, 256)])  # partial slot last
            a_sl = {}

            def emit_slot(slot):
                ps = pssc.tile([128, QC], F32, tag="ssc")
                a = apool.tile([128, QC], BF16, tag="a")
                soff = 0
                for kt, qb, w in slot:
                    nc.tensor.matmul(
                        ps[:, soff : soff + w],
                        lhsT=kT[b][:, kt * 128 : (kt + 1) * 128],
                        rhs=qT[b][j][:, qc * QC + qb : (qc + 1) * QC],
                        start=True,
                        stop=True,
                    )
                    a_sl[kt] = (a, soff, qb)
                    soff += w
                nc.scalar.activation(
                    out=a[:, 0:soff], in_=ps[:, 0:soff], func=EXP, scale=SCALE
                )
                for kt, qb, w in slot:
                    if kt >= d0:  # diagonal: triangular mask
                        ao = a_sl[kt][1]
                        nc.vector.tensor_mul(
                            a[:, ao : ao + 128], a[:, ao : ao + 128], mask_sb
                        )

            ct = cpool.tile([128, QC], BF16, tag="ct", bufs=12)

            def emit_ctx(st):
                qt = qc * 4 + st
                cps = pss.tile([128, HD + 1], F32, tag="small")
                for kt in range(qt + 1):
                    a, ao, qb = a_sl[kt]
                    nc.tensor.matmul(
                        cps,
                        lhsT=a[:, ao + st * 128 - qb : ao + (st + 1) * 128 - qb],
                        rhs=vext[b][kt],
                        start=(kt == 0),
                        stop=(kt == qt),
                    )
                zr = cpool.tile([128, 1], F32, tag="zr", bufs=6)
                nc.vector.reciprocal(zr, cps[:, HD : HD + 1])
                cs = cpool.tile([128, HD], BF16, tag="cs", bufs=6)
                nc.vector.tensor_scalar_mul(cs, cps[:, 0:HD], zr)

                def fin(st=st, cs=cs):
                    tp = pst.tile([128, 128], BF16, tag="tp")
                    nc.tensor.transpose(tp, cs, ident)
                    nc.vector.tensor_copy(ct[:, st * 128 : (st + 1) * 128], tp)

                push_fin(fin)

            def emit_dma():
                if j == 0:
                    last_ct_ref[0] = nc.sync.dma_start(
                        out=ct_dram0[4 * b + qc], in_=ct
                    )
                else:
                    last_ct_ref[0] = nc.sync.dma_start(
                        out=ct_dram1[4 * b + qc], in_=ct
                    )

            # slots 0..d0+1 cover k-tiles d0, d0+1, d0+3 -> st 0,1,3 ready;
            # the final slot only adds d0+2 (needed by st 2,3)
            for slot in slots[:-1]:
                emit_slot(slot)
            emit_ctx(0)
            emit_ctx(1)
            emit_slot(slots[-1])
            emit_ctx(2)
            emit_ctx(3)
            push_fin(emit_dma)

        # ===== Phase 1 + interleaved j=0 attention (one chunk late, so each
        # unit's exp has a full projection chunk of slack before its ctx
        # matmuls reach the head of the in-order PE queue) =====
        with tc.tile_pool(name="x", bufs=2) as xpool:

            def proj_chunk(b, qc):
                xstrip = xpool.tile([128, NDC, QC], BF16, tag="xs")
                if b == 0 and qc == 0:
                    for dq in range(4):
                        nc.sync.dma_start(
                            out=xstrip[:, 4 * dq : 4 * (dq + 1), :],
                            in_=xq[b, qc, 4 * dq : 4 * (dq + 1)].rearrange(
                                "a p q -> p a q"
                            ),
                        )
                else:
                    strip_dmas.append(
                        nc.sync.dma_start(
                            out=xstrip, in_=xq[b, qc].rearrange("a p q -> p a q")
                        )
                    )
                xs = [xstrip[:, dc, :] for dc in range(NDC)]
                # Q^T per head-slot in its own 1-bank PSUM tile
                for j in range(2):
                    psq = pssc.tile([128, QC], F32, tag="ssc")
                    for dc in range(NDC):
                        nc.tensor.matmul(
                            psq,
                            lhsT=wq_sb[dc][:, j * HD : (j + 1) * HD],
                            rhs=xs[dc],
                            start=(dc == 0),
                            stop=(dc == NDC - 1),
                        )
                    nc.vector.tensor_copy(
                        qT[b][j][:, qc * QC : (qc + 1) * QC], psq
                    )
                # K^T: [dh, q 512]
                psk = pssc.tile([128, QC], F32, tag="ssc")
                for dc in range(NDC):
                    nc.tensor.matmul(
                        psk,
                        lhsT=wk_sb[dc],
                        rhs=xs[dc],
                        start=(dc == 0),
                        stop=(dc == NDC - 1),
                    )
                nc.vector.tensor_copy(kT[b][:, qc * QC : (qc + 1) * QC], psk)
                # V: [s-tile 128, dv 128] (natural orientation)
                for st in range(4):
                    kt = qc * 4 + st
                    ps = pss.tile([128, HD + 1], F32, tag="small")
                    for dc in range(NDC):
                        nc.tensor.matmul(
                            ps[:, 0:HD],
                            lhsT=xs[dc][:, st * 128 : (st + 1) * 128],
                            rhs=wv_sb[dc],
                            start=(dc == 0),
                            stop=(dc == NDC - 1),
                        )
                    nc.vector.tensor_copy(vext[b][kt][:, 0:HD], ps[:, 0:HD])
                    nc.vector.memset(vext[b][kt][:, HD : HD + 1], 1.0)

            strip_dmas = []
            chunks = [(b, qc) for b in range(B) for qc in range(NQC)]
            # j=1 units pulled into phase 1 (ACT has headroom there, while
            # the j-1-only phase 2 is exp/Activation-bound): after chunk i,
            # run the listed (b, qc) j=1 unit.
            j1_early = {2: (0, 0), 3: (0, 1), 5: (1, 0), 6: (1, 1), 7: (0, 2)}
            for i, (b, qc) in enumerate(chunks):
                proj_chunk(b, qc)
                if b == 0 and qc == 3:
                    # defer the big wo load until the x strips have a
                    # head start; it only matters in phase 3
                    wo_dma = nc.gpsimd.dma_start(
                        out=woall, in_=wo.rearrange("a p q -> p a q")
                    )
                    add_dep_helper(
                        wo_dma.ins, strip_dmas[-1].ins, reason="yield-to-strips"
                    )
                if i >= 1:
                    pb, pqc = chunks[i - 1]
                    attn_unit(pb, 0, pqc)
                if i in j1_early:
                    eb, eqc = j1_early[i]
                    attn_unit(eb, 1, eqc)
            attn_unit(chunks[-1][0], 0, chunks[-1][1])
            flush_fins()

        coll0 = None
        if not sim:
            coll0 = nc.gpsimd.collective_compute(
                "AllToAll",
                mybir.AluOpType.bypass,
                replica_groups=REPLICA_GROUPS,
                ins=[ct_dram0[:, :, :].opt()],
                outs=[recv0[:, :, :].opt()],
            )

        # ===== remaining j=1 attention (big units first; a medium one last
        # so the second collective's latency tail is short) =====
        for b, qc in ((0, 3), (1, 3), (1, 2)):
            attn_unit(b, 1, qc)
        flush_fins()
        last_ct = last_ct_ref[0]
        coll1 = None
        if not sim:
            coll1 = nc.gpsimd.collective_compute(
                "AllToAll",
                mybir.AluOpType.bypass,
                replica_groups=REPLICA_GROUPS,
                ins=[ct_dram1[:, :, :].opt()],
                outs=[recv1[:, :, :].opt()],
            )

        # ===== Phase 3: row-parallel output projection, two waves =====
        accp = es.enter_context(tc.tile_pool(name="acc", bufs=1))
        accs = [
            accp.tile([128, D], F32, tag=f"acc{st}", name=f"r{_rep}acc{st}")
            for st in range(4)
        ]
        cstrips = []
        src0 = ct_dram0 if sim else recv0
        src1 = ct_dram1 if sim else recv1
        cstrip0 = spool.tile([128, N_CORES, QC], BF16, tag="cstrip0")
        d = nc.sync.dma_start(
            out=cstrip0, in_=src0[:, :, :].rearrange("g p q -> p g q")
        )
        if not sim:
            add_dep_helper(d.ins, coll0.ins, reason="alltoall->read")
            # keep the scheduler from hoisting this read ahead of the j=1 ct
            # writes on the SP queue (it would chain them behind coll0)
            add_dep_helper(d.ins, last_ct.ins, reason="order-after-ct-writes")
        cstrips.append(cstrip0)
        cstrip1 = spool.tile([128, N_CORES, QC], BF16, tag="cstrip1")
        d = nc.sync.dma_start(
            out=cstrip1, in_=src1[:, :, :].rearrange("g p q -> p g q")
        )
        if not sim:
            add_dep_helper(d.ins, coll1.ins, reason="alltoall->read")
            add_dep_helper(d.ins, last_ct.ins, reason="order-after-ct-writes")
        cstrips.append(cstrip1)

        units = [(st, cc) for st in range(4) for cc in range(4)]
        unitsB = [(st, cc) for st in (0, 1, 2, 3) for cc in range(4)]

        def wave(j, final):
            ulist = unitsB if final else units
            for u, (st, cc) in enumerate(ulist):
                half = pssc.tile([128, QC], F32, tag="ssc")
                if not final:
                    nc.tensor.matmul(
                        half,
                        lhsT=ones1,
                        rhs=bo_sb[:, cc * QC : (cc + 1) * QC],
                        start=True,
                        stop=False,
                    )
                for s in range(N_CORES):
                    dch = 4 * (s % 4) + 2 * (s // 4) + j
                    nc.tensor.matmul(
                        half,
                        lhsT=cstrips[j][:, s, st * 128 : (st + 1) * 128],
                        rhs=woall[:, dch, cc * QC : (cc + 1) * QC],
                        start=(final and s == 0),
                        stop=(s == N_CORES - 1),
                    )
                if not final:
                    if u % 2 == 0:
                        nc.vector.tensor_copy(
                            accs[st][:, cc * QC : (cc + 1) * QC], half
                        )
                    else:
                        nc.scalar.activation(
                            out=accs[st][:, cc * QC : (cc + 1) * QC],
                            in_=half,
                            func=CP,
                        )
                else:
                    osb = opool.tile([128, QC], BF16, tag="osb")
                    nc.vector.tensor_add(
                        osb, half, accs[st][:, cc * QC : (cc + 1) * QC]
                    )
                    nc.sync.dma_start(
                        out=out_ext[
                            st * 128 : (st + 1) * 128, cc * QC : (cc + 1) * QC
                        ],
                        in_=osb,
                    )

        wave(0, final=False)
        wave(1, final=True)


def _make_mask() -> np.ndarray:
    # mask[k, q] = 1.0 if q >= k (triangular causal for the [128,128]
    # diagonal sub-block of each diagonal k-tile)
    q = np.arange(128)[None, :]
    k = np.arange(128)[:, None]
    return (q >= k).astype(ml_dtypes.bfloat16)


def _make_in_maps(inputs) -> list[dict]:
    x = np.asarray(inputs["x"], dtype=np.float32)
    Wq = np.asarray(inputs["Wq"], dtype=np.float32)
    Wk = np.asarray(inputs["Wk"], dtype=np.float32)
    Wv = np.asarray(inputs["Wv"], dtype=np.float32)
    Wo = np.asarray(inputs["Wo"], dtype=np.float32)
    bo = np.asarray(inputs["bo"], dtype=np.float32)

    bf = ml_dtypes.bfloat16
    mask = _make_mask()

    # x^T tiled: [b, qc, dc, 128, 512], both batches shipped to every core
    xqs = []
    for b in range(B):
        xT = np.ascontiguousarray(x[b].T.astype(bf))  # [d, s]
        xqs.append(xT.reshape(NDC, 128, NQC, QC).transpose(2, 0, 1, 3))
    xq_all = np.ascontiguousarray(np.stack(xqs))

    wo_full = np.ascontiguousarray(Wo.astype(bf).reshape(NDC, 128, D))
    bo_full = np.ascontiguousarray(bo.astype(bf).reshape(1, D))

    in_maps = []
    for c in range(N_CORES):
        g, half = c % 4, c // 4
        q_lo = g * 512 + half * 2 * HD  # this core's two head-slots of group g
        in_maps.append(
            {
                "xq": xq_all,
                "wq": np.ascontiguousarray(
                    Wq[:, q_lo : q_lo + 2 * HD].astype(bf).reshape(NDC, 128, 2 * HD)
                ),
                "wk": np.ascontiguousarray(
                    Wk[:, g * HD : (g + 1) * HD].astype(bf).reshape(NDC, 128, HD)
                ),
                "wv": np.ascontiguousarray(
                    Wv[:, g * HD : (g + 1) * HD].astype(bf).reshape(NDC, 128, HD)
                ),
                "wo": wo_full,
                "bo": bo_full,
                "msk": mask,
            }
        )
    return in_maps


def _assemble(results) -> np.ndarray:
    out = np.empty((B, S, D), dtype=np.float32)
    for c in range(N_CORES):
        b, r = c // 4, c % 4
        out[b][r * QC : (r + 1) * QC, :] = results[c]["out"].astype(np.float32)
    return out


def kernel(**inputs) -> np.ndarray:
    in_maps = _make_in_maps(inputs)
    nc = _build_program()
    res = run_bass_kernel_spmd(nc, in_maps, list(range(N_CORES)))
    return _assemble(res.results)



# revision 29
# speedup vs baseline: 1.1027x; 1.0222x over previous
"""GroupedQueryAttention Trainium2 kernel (8 NeuronCores).

Sharding: core c -> (kv-group g = c%4, head-slot pair {2*(c//4), 2*(c//4)+1}).
Each core computes its group's two head-slots over BOTH batches (K/V are
computed per batch on every core), then two 8-rank AllToAlls (one per
head-slot j) redistribute ctx^T so core c ends up with ctx^T of ALL 16
heads for its own flat row chunk c (batch c//4, rows 512*(c%4)..+512).
The output projection is then row-parallel with the full Wo resident —
no further collectives.

Every AllToAll block is useful: src s = (g=s%4, half=s//4) contributes
head (g, 2*half+j) of batch d//4, rows 512*(d%4).. to dest d, and the
d_model chunk index of block (s, j) is the compile-time constant
4*(s%4) + 2*(s//4) + j — pure SPMD, no rank-dependent indexing.

Pipelining: the j=0 attention units are emitted interleaved with the
projection chunks so their exp runs on the otherwise-idle Activation
engine during phase 1, letting the first AllToAll issue right after the
projections drain. The output projection runs in two waves: bias + j=0
terms accumulate into an SBUF fp32 accumulator while the second
AllToAll is still in flight; the j=1 terms and the final add/store run
after it lands.

Layout trick: scores are computed transposed (S^T[k, q]) so A^T =
exp(S^T) is directly the lhsT of the ctx matmul. The softmax denominator
comes free as a 129th "ones" column appended to V; ctx rows are
normalized by a per-partition reciprocal scale. Score blocks are packed
into 512-col PSUM bank slots with fully-masked diagonal columns trimmed,
so exp reads one contiguous range per [128,1024] tile.
"""

from contextlib import ExitStack

import numpy as np
import ml_dtypes

import concourse.bass as bass
import concourse.bacc as bacc
import concourse.tile as tile
from concourse import mybir
from concourse.bass_utils import run_bass_kernel_spmd
from concourse.masks import make_identity
from concourse.tile_rust import add_dep_helper

BF16 = mybir.dt.bfloat16
F32 = mybir.dt.float32

B = 2
S = 2048
D = 2048
G = 4  # kv groups
HPG = 4  # heads per group
HD = 128  # head dim
QC = 512  # q-chunk (columns per S^T block)
NQC = S // QC  # 4
NKT = S // 128  # 16 k-tiles
NDC = D // 128  # 16 d_in chunks
SCALE = 1.0 / float(np.sqrt(HD))
N_CORES = 8
REPLICA_GROUPS = [[0, 1, 2, 3, 4, 5, 6, 7]]

CP = mybir.ActivationFunctionType.Copy
EXP = mybir.ActivationFunctionType.Exp


def _build_program(repeat: int = 1, sim: bool = False):
    nc = bacc.Bacc("TRN2", target_bir_lowering=False, debug=False)

    xq = nc.declare_dram_parameter("xq", [B, NQC, NDC, 128, QC], BF16, isOutput=False)
    wq = nc.declare_dram_parameter("wq", [NDC, 128, 2 * HD], BF16, isOutput=False)
    wk = nc.declare_dram_parameter("wk", [NDC, 128, HD], BF16, isOutput=False)
    wv = nc.declare_dram_parameter("wv", [NDC, 128, HD], BF16, isOutput=False)
    wo = nc.declare_dram_parameter("wo", [NDC, 128, D], BF16, isOutput=False)
    bo = nc.declare_dram_parameter("bo", [1, D], BF16, isOutput=False)
    msk = nc.declare_dram_parameter("msk", [128, 128], BF16, isOutput=False)
    out_ext = nc.declare_dram_parameter("out", [QC, D], BF16, isOutput=True)

    # AllToAll outputs: recv[j] block s = ctx^T of (g=s%4, h=2*(s//4)+j)
    # for this core's flat row chunk
    recv0 = nc.dram_tensor("recv0", [N_CORES, HD, QC], BF16)
    recv1 = nc.dram_tensor("recv1", [N_CORES, HD, QC], BF16)

    for _rep in range(repeat):
        _build_body(nc, _rep, xq, wq, wk, wv, wo, bo, msk, out_ext, recv0, recv1,
                    sim=sim)

    nc.compile()
    return nc


def _build_sim_program():
    """Single-core, collective-free variant of the body for TimelineSim."""
    nc = bacc.Bacc("TRN2", target_bir_lowering=False, debug=False)
    xq = nc.declare_dram_parameter("xq", [B, NQC, NDC, 128, QC], BF16, isOutput=False)
    wq = nc.declare_dram_parameter("wq", [NDC, 128, 2 * HD], BF16, isOutput=False)
    wk = nc.declare_dram_parameter("wk", [NDC, 128, HD], BF16, isOutput=False)
    wv = nc.declare_dram_parameter("wv", [NDC, 128, HD], BF16, isOutput=False)
    wo = nc.declare_dram_parameter("wo", [NDC, 128, D], BF16, isOutput=False)
    bo = nc.declare_dram_parameter("bo", [1, D], BF16, isOutput=False)
    msk = nc.declare_dram_parameter("msk", [128, 128], BF16, isOutput=False)
    out_ext = nc.declare_dram_parameter("out", [QC, D], BF16, isOutput=True)
    recv0 = nc.dram_tensor("recv0", [N_CORES, HD, QC], BF16)
    recv1 = nc.dram_tensor("recv1", [N_CORES, HD, QC], BF16)
    _build_body(nc, 0, xq, wq, wk, wv, wo, bo, msk, out_ext, recv0, recv1, sim=True)
    nc.compile()
    return nc


def _build_body(nc, _rep, xq, wq, wk, wv, wo, bo, msk, out_ext, recv0, recv1,
                sim=False):
    with tile.TileContext(nc) as tc, ExitStack() as es:
        singles = es.enter_context(tc.tile_pool(name="singles", bufs=1))
        wpool = es.enter_context(tc.tile_pool(name="w", bufs=1))
        qkpool = es.enter_context(tc.tile_pool(name="qk", bufs=1))
        apool = es.enter_context(tc.tile_pool(name="a", bufs=24))
        spool = es.enter_context(tc.tile_pool(name="sm", bufs=1))
        cpool = es.enter_context(tc.tile_pool(name="cs", bufs=6))
        opool = es.enter_context(tc.tile_pool(name="ob", bufs=2))
        pssc = es.enter_context(tc.tile_pool(name="pssc", bufs=4, space="PSUM"))
        pss = es.enter_context(tc.tile_pool(name="pss", bufs=2, space="PSUM"))
        pst = es.enter_context(tc.tile_pool(name="pst", bufs=2, space="PSUM"))
        dram = es.enter_context(tc.tile_pool(name="dram", bufs=1, space="DRAM"))
        dram2 = es.enter_context(tc.tile_pool(name="dram2", bufs=1, space="DRAM"))

        # --- constants (off the SP queue so x strip 0 starts immediately) ---
        ident = singles.tile([128, 128], BF16, tag="ident")
        make_identity(nc, ident)
        ones1 = singles.tile([1, 128], BF16, tag="ones1")
        nc.vector.memset(ones1, 1.0)
        bias_bc = singles.tile([128, D], BF16, tag="bias_bc")
        nc.scalar.dma_start(out=bias_bc, in_=bo[:, :].broadcast_to([128, D]))
        mask_sb = singles.tile([128, 128], BF16, tag="mask")
        nc.scalar.dma_start(out=mask_sb, in_=msk[:, :])
        # preload the exp activation table so phase 2's first exp is cheap
        warm = singles.tile([1, 4], F32, tag="warm")
        nc.scalar.activation(out=warm, in_=ones1[:, 0:4], func=EXP)

        # --- resident weights, loaded on the gpsimd queue (idle early).
        # wq is split so the first Q matmuls (dc 0..3) start sooner; wo
        # (8.4 MB, ~23 us of DMA) is deferred until the x strips are ahead
        # of the PE (it is only needed in phase 3) to avoid starving the
        # strip loads mid-phase-1.
        wqall = wpool.tile([128, NDC, 2 * HD], BF16, tag="wqall")
        nc.gpsimd.dma_start(
            out=wqall[:, 0:4, :], in_=wq[0:4].rearrange("a p q -> p a q")
        )
        nc.gpsimd.dma_start(
            out=wqall[:, 4:NDC, :], in_=wq[4:NDC].rearrange("a p q -> p a q")
        )
        wkall = wpool.tile([128, NDC, HD], BF16, tag="wkall")
        nc.gpsimd.dma_start(out=wkall, in_=wk.rearrange("a p q -> p a q"))
        wvall = wpool.tile([128, NDC, HD], BF16, tag="wvall")
        nc.gpsimd.dma_start(out=wvall, in_=wv.rearrange("a p q -> p a q"))
        woall = wpool.tile([128, NDC, D], BF16, tag="woall")
        wq_sb = [wqall[:, dc, :] for dc in range(NDC)]
        wk_sb = [wkall[:, dc, :] for dc in range(NDC)]
        wv_sb = [wvall[:, dc, :] for dc in range(NDC)]

        # --- persistent activations (per batch) ---
        qT = [
            [
                qkpool.tile([128, S], BF16, tag=f"qT{b}{j}", name=f"r{_rep}qT{b}{j}")
                for j in range(2)
            ]
            for b in range(B)
        ]
        kT = [
            qkpool.tile([128, S], BF16, tag=f"kT{b}", name=f"r{_rep}kT{b}") for b in range(B)
        ]
        vext = [
            [
                qkpool.tile([128, HD + 1], BF16, tag=f"v{b}_{i}", name=f"r{_rep}v{b}_{i}")
                for i in range(NKT)
            ]
            for b in range(B)
        ]

        ct_dram0 = dram.tile([N_CORES, HD, QC], BF16, tag="ct0", name=f"r{_rep}ctd0")
        ct_dram1 = dram2.tile([N_CORES, HD, QC], BF16, tag="ct1", name=f"r{_rep}ctd1")

        # Deferred-finish queue: the transpose (PE) + copy (DVE) tail of each
        # ctx tile is emitted LAG ctx-tiles late, so by the time the PE
        # in-order queue reaches a transpose its DVE-normalized input is
        # already done and the PE never stalls on the DVE chain.
        fin_q = []
        FIN_LAG = 2
        last_ct_ref = [None]

        def push_fin(f):
            fin_q.append(f)
            while len(fin_q) > FIN_LAG:
                fin_q.pop(0)()

        def flush_fins():
            while fin_q:
                fin_q.pop(0)()

        def attn_unit(b, j, qc):
            """Attention for (batch b, head-slot j, q-chunk qc); writes
            ctx^T [128, 512] to ct_dram[j] block 4*b+qc."""
            d0 = 4 * qc
            # 512-col PSUM bank slots; diagonal tiles trimmed to live cols.
            # Each slot gets its own 1-bank PSUM tile + exp, so a score
            # matmul only WAR-waits on the exp from 4 slots ago (not the
            # 1024-wide exp of the previous slot pair).
            slots = [[(kt, 0, QC)] for kt in range(d0)]
            slots.append([(d0, 0, QC)])
            slots.append([(d0 + 1, 128, 384), (d0 + 3, 384, 128)])
            slots.append([(d0 + 2, 256, 256)])  # partial slot last
            a_sl = {}

            def emit_slot(slot):
                ps = pssc.tile([128, QC], F32, tag="ssc")
                a = apool.tile([128, QC], BF16, tag="a")
                soff = 0
                for kt, qb, w in slot:
                    nc.tensor.matmul(
                        ps[:, soff : soff + w],
                        lhsT=kT[b][:, kt * 128 : (kt + 1) * 128],
                        rhs=qT[b][j][:, qc * QC + qb : (qc + 1) * QC],
                        start=True,
                        stop=True,
                    )
                    a_sl[kt] = (a, soff, qb)
                    soff += w
                nc.scalar.activation(
                    out=a[:, 0:soff], in_=ps[:, 0:soff], func=EXP, scale=SCALE
                )
                for kt, qb, w in slot:
                    if kt >= d0:  # diagonal: triangular mask
                        ao = a_sl[kt][1]
                        nc.vector.tensor_mul(
                            a[:, ao : ao + 128], a[:, ao : ao + 128], mask_sb
                        )

            ct = cpool.tile([128, QC], BF16, tag="ct", bufs=12)

            def emit_ctx(st):
                qt = qc * 4 + st
                cps = pss.tile([128, HD + 1], F32, tag="small")
                for kt in range(qt + 1):
                    a, ao, qb = a_sl[kt]
                    nc.tensor.matmul(
                        cps,
                        lhsT=a[:, ao + st * 128 - qb : ao + (st + 1) * 128 - qb],
                        rhs=vext[b][kt],
                        start=(kt == 0),
                        stop=(kt == qt),
                    )
                zr = cpool.tile([128, 1], F32, tag="zr", bufs=6)
                nc.vector.reciprocal(zr, cps[:, HD : HD + 1])
                cs = cpool.tile([128, HD], BF16, tag="cs", bufs=6)
                nc.vector.tensor_scalar_mul(cs, cps[:, 0:HD], zr)

                def fin(st=st, cs=cs):
                    tp = pst.tile([128, 128], BF16, tag="tp")
                    nc.tensor.transpose(tp, cs, ident)
                    nc.vector.tensor_copy(ct[:, st * 128 : (st + 1) * 128], tp)

                push_fin(fin)

            def emit_dma():
                if j == 0:
                    last_ct_ref[0] = nc.sync.dma_start(
                        out=ct_dram0[4 * b + qc], in_=ct
                    )
                else:
                    last_ct_ref[0] = nc.sync.dma_start(
                        out=ct_dram1[4 * b + qc], in_=ct
                    )

            # slots 0..d0+1 cover k-tiles d0, d0+1, d0+3 -> st 0,1,3 ready;
            # the final slot only adds d0+2 (needed by st 2,3)
            for slot in slots[:-1]:
                emit_slot(slot)
            emit_ctx(0)
            emit_ctx(1)
            emit_slot(slots[-1])
            emit_ctx(2)
            emit_ctx(3)
            push_fin(emit_dma)

        # ===== Phase 1 + interleaved j=0 attention (one chunk late, so each
        # unit's exp has a full projection chunk of slack before its ctx
        # matmuls reach the head of the in-order PE queue) =====
        with tc.tile_pool(name="x", bufs=2) as xpool:

            def proj_chunk(b, qc):
                xstrip = xpool.tile([128, NDC, QC], BF16, tag="xs")
                if b == 0 and qc == 0:
                    for dq in range(4):
                        nc.sync.dma_start(
                            out=xstrip[:, 4 * dq : 4 * (dq + 1), :],
                            in_=xq[b, qc, 4 * dq : 4 * (dq + 1)].rearrange(
                                "a p q -> p a q"
                            ),
                        )
                else:
                    strip_dmas.append(
                        nc.sync.dma_start(
                            out=xstrip, in_=xq[b, qc].rearrange("a p q -> p a q")
                        )
                    )
                xs = [xstrip[:, dc, :] for dc in range(NDC)]
                # Q^T per head-slot in its own 1-bank PSUM tile
                for j in range(2):
                    psq = pssc.tile([128, QC], F32, tag="ssc")
                    for dc in range(NDC):
                        nc.tensor.matmul(
                            psq,
                            lhsT=wq_sb[dc][:, j * HD : (j + 1) * HD],
                            rhs=xs[dc],
                            start=(dc == 0),
                            stop=(dc == NDC - 1),
                        )
                    nc.vector.tensor_copy(
                        qT[b][j][:, qc * QC : (qc + 1) * QC], psq
                    )
                # K^T: [dh, q 512]
                psk = pssc.tile([128, QC], F32, tag="ssc")
                for dc in range(NDC):
                    nc.tensor.matmul(
                        psk,
                        lhsT=wk_sb[dc],
                        rhs=xs[dc],
                        start=(dc == 0),
                        stop=(dc == NDC - 1),
                    )
                nc.vector.tensor_copy(kT[b][:, qc * QC : (qc + 1) * QC], psk)
                # V: [s-tile 128, dv 128] (natural orientation)
                for st in range(4):
                    kt = qc * 4 + st
                    ps = pss.tile([128, HD + 1], F32, tag="small")
                    for dc in range(NDC):
                        nc.tensor.matmul(
                            ps[:, 0:HD],
                            lhsT=xs[dc][:, st * 128 : (st + 1) * 128],
                            rhs=wv_sb[dc],
                            start=(dc == 0),
                            stop=(dc == NDC - 1),
                        )
                    nc.vector.tensor_copy(vext[b][kt][:, 0:HD], ps[:, 0:HD])
                    nc.vector.memset(vext[b][kt][:, HD : HD + 1], 1.0)

            strip_dmas = []
            chunks = [(b, qc) for b in range(B) for qc in range(NQC)]
            # j=1 units pulled into phase 1 (ACT has headroom there, while
            # the j-1-only phase 2 is exp/Activation-bound): after chunk i,
            # run the listed (b, qc) j=1 unit.
            j1_early = {2: (0, 0), 3: (0, 1), 5: (1, 0), 6: (1, 1), 7: (0, 2)}
            for i, (b, qc) in enumerate(chunks):
                proj_chunk(b, qc)
                if b == 0 and qc == 3:
                    # defer the big wo load until the x strips have a
                    # head start; it only matters in phase 3
                    wo_dma = nc.gpsimd.dma_start(
                        out=woall, in_=wo.rearrange("a p q -> p a q")
                    )
                    add_dep_helper(
                        wo_dma.ins, strip_dmas[-1].ins, reason="yield-to-strips"
                    )
                if i >= 1:
                    pb, pqc = chunks[i - 1]
                    attn_unit(pb, 0, pqc)
                if i in j1_early:
                    eb, eqc = j1_early[i]
                    attn_unit(eb, 1, eqc)
            attn_unit(chunks[-1][0], 0, chunks[-1][1])
            flush_fins()

        coll0 = None
        if not sim:
            coll0 = nc.gpsimd.collective_compute(
                "AllToAll",
                mybir.AluOpType.bypass,
                replica_groups=REPLICA_GROUPS,
                ins=[ct_dram0[:, :, :].opt()],
                outs=[recv0[:, :, :].opt()],
            )

        # ===== remaining j=1 attention (big units first; a medium one last
        # so the second collective's latency tail is short) =====
        for b, qc in ((0, 3), (1, 3), (1, 2)):
            attn_unit(b, 1, qc)
        flush_fins()
        last_ct = last_ct_ref[0]
        coll1 = None
        if not sim:
            coll1 = nc.gpsimd.collective_compute(
                "AllToAll",
                mybir.AluOpType.bypass,
                replica_groups=REPLICA_GROUPS,
                ins=[ct_dram1[:, :, :].opt()],
                outs=[recv1[:, :, :].opt()],
            )

        # ===== Phase 3: row-parallel output projection, two waves =====
        accp = es.enter_context(tc.tile_pool(name="acc", bufs=1))
        accs = [
            accp.tile([128, D], F32, tag=f"acc{st}", name=f"r{_rep}acc{st}")
            for st in range(4)
        ]
        cstrips = []
        src0 = ct_dram0 if sim else recv0
        src1 = ct_dram1 if sim else recv1
        cstrip0 = spool.tile([128, N_CORES, QC], BF16, tag="cstrip0")
        d = nc.sync.dma_start(
            out=cstrip0, in_=src0[:, :, :].rearrange("g p q -> p g q")
        )
        if not sim:
            add_dep_helper(d.ins, coll0.ins, reason="alltoall->read")
            # keep the scheduler from hoisting this read ahead of the j=1 ct
            # writes on the SP queue (it would chain them behind coll0)
            add_dep_helper(d.ins, last_ct.ins, reason="order-after-ct-writes")
        cstrips.append(cstrip0)
        cstrip1 = spool.tile([128, N_CORES, QC], BF16, tag="cstrip1")
        d = nc.sync.dma_start(
            out=cstrip1, in_=src1[:, :, :].rearrange("g p q -> p g q")
        )
        if not sim:
            add_dep_helper(d.ins, coll1.ins, reason="alltoall->read")
            add_dep_helper(d.ins, last_ct.ins, reason="order-after-ct-writes")
        cstrips.append(cstrip1)

        units = [(st, cc) for st in range(4) for cc in range(4)]
        unitsB = [(st, cc) for st in (0, 1, 2, 3) for cc in range(4)]

        def wave(j, final):
            ulist = unitsB if final else units
            for u, (st, cc) in enumerate(ulist):
                half = pssc.tile([128, QC], F32, tag="ssc")
                for s in range(N_CORES):
                    dch = 4 * (s % 4) + 2 * (s // 4) + j
                    nc.tensor.matmul(
                        half,
                        lhsT=cstrips[j][:, s, st * 128 : (st + 1) * 128],
                        rhs=woall[:, dch, cc * QC : (cc + 1) * QC],
                        start=(s == 0),
                        stop=(s == N_CORES - 1),
                    )
                if not final:
                    nc.vector.tensor_add(
                        accs[st][:, cc * QC : (cc + 1) * QC],
                        half,
                        bias_bc[:, cc * QC : (cc + 1) * QC],
                    )
                else:
                    osb = opool.tile([128, QC], BF16, tag="osb")
                    nc.vector.tensor_add(
                        osb, half, accs[st][:, cc * QC : (cc + 1) * QC]
                    )
                    nc.sync.dma_start(
                        out=out_ext[
                            st * 128 : (st + 1) * 128, cc * QC : (cc + 1) * QC
                        ],
                        in_=osb,
                    )

        wave(0, final=False)
        wave(1, final=True)


def _make_mask() -> np.ndarray:
    # mask[k, q] = 1.0 if q >= k (triangular causal for the [128,128]
    # diagonal sub-block of each diagonal k-tile)
    q = np.arange(128)[None, :]
    k = np.arange(128)[:, None]
    return (q >= k).astype(ml_dtypes.bfloat16)


def _make_in_maps(inputs) -> list[dict]:
    x = np.asarray(inputs["x"], dtype=np.float32)
    Wq = np.asarray(inputs["Wq"], dtype=np.float32)
    Wk = np.asarray(inputs["Wk"], dtype=np.float32)
    Wv = np.asarray(inputs["Wv"], dtype=np.float32)
    Wo = np.asarray(inputs["Wo"], dtype=np.float32)
    bo = np.asarray(inputs["bo"], dtype=np.float32)

    bf = ml_dtypes.bfloat16
    mask = _make_mask()

    # x^T tiled: [b, qc, dc, 128, 512], both batches shipped to every core
    xqs = []
    for b in range(B):
        xT = np.ascontiguousarray(x[b].T.astype(bf))  # [d, s]
        xqs.append(xT.reshape(NDC, 128, NQC, QC).transpose(2, 0, 1, 3))
    xq_all = np.ascontiguousarray(np.stack(xqs))

    wo_full = np.ascontiguousarray(Wo.astype(bf).reshape(NDC, 128, D))
    bo_full = np.ascontiguousarray(bo.astype(bf).reshape(1, D))

    in_maps = []
    for c in range(N_CORES):
        g, half = c % 4, c // 4
        q_lo = g * 512 + half * 2 * HD  # this core's two head-slots of group g
        in_maps.append(
            {
                "xq": xq_all,
                "wq": np.ascontiguousarray(
                    Wq[:, q_lo : q_lo + 2 * HD].astype(bf).reshape(NDC, 128, 2 * HD)
                ),
                "wk": np.ascontiguousarray(
                    Wk[:, g * HD : (g + 1) * HD].astype(bf).reshape(NDC, 128, HD)
                ),
                "wv": np.ascontiguousarray(
                    Wv[:, g * HD : (g + 1) * HD].astype(bf).reshape(NDC, 128, HD)
                ),
                "wo": wo_full,
                "bo": bo_full,
                "msk": mask,
            }
        )
    return in_maps


def _assemble(results) -> np.ndarray:
    out = np.empty((B, S, D), dtype=np.float32)
    for c in range(N_CORES):
        b, r = c // 4, c % 4
        out[b][r * QC : (r + 1) * QC, :] = results[c]["out"].astype(np.float32)
    return out


def kernel(**inputs) -> np.ndarray:
    in_maps = _make_in_maps(inputs)
    nc = _build_program()
    res = run_bass_kernel_spmd(nc, in_maps, list(range(N_CORES)))
    return _assemble(res.results)



# revision 31
# speedup vs baseline: 12.7147x; 11.5309x over previous
"""GroupedQueryAttention Trainium2 kernel (8 NeuronCores).

Sharding: core c -> (kv-group g = c%4, head-slot pair {2*(c//4), 2*(c//4)+1}).
Each core computes its group's two head-slots over BOTH batches (K/V are
computed per batch on every core), then two 8-rank AllToAlls (one per
head-slot j) redistribute ctx^T so core c ends up with ctx^T of ALL 16
heads for its own flat row chunk c (batch c//4, rows 512*(c%4)..+512).
The output projection is then row-parallel with the full Wo resident —
no further collectives.

Every AllToAll block is useful: src s = (g=s%4, half=s//4) contributes
head (g, 2*half+j) of batch d//4, rows 512*(d%4).. to dest d, and the
d_model chunk index of block (s, j) is the compile-time constant
4*(s%4) + 2*(s//4) + j — pure SPMD, no rank-dependent indexing.

Pipelining: the attention phase is Activation-engine-bound (each
512-col exp costs ~610 ns vs ~530 ns of PE work per score slot), so
attention units are spread into the projection phase, which has ACT
headroom: every j=0 unit runs one chunk behind its projections, and
five of the eight j=1 units are pulled forward too (j1_early).  Only
three j=1 units remain after the first AllToAll, which therefore
issues early and hides; the second AllToAll is covered by wave 0 of
the output projection (j=0 terms + broadcast bias accumulate into an
SBUF fp32 accumulator), with the j=1 terms and the bf16 add/store in
wave 1 after it lands.

Layout tricks: scores are computed transposed (S^T[k, q]) so A^T =
exp(S^T) is directly the lhsT of the ctx matmul.  The softmax
denominator comes free as a 129th "ones" column appended to V; ctx
rows are normalized by a per-partition reciprocal scale.  Each score
slot owns a single 1-bank [128,512] PSUM tile with its own exp, so a
score matmul only WAR-waits on the exp four slots back.  All
PSUM->SBUF projection copies run on the DVE (not ACT), the bias is
materialized once by a stride-0 broadcast DMA (no per-tile bias
matmuls), the ctx transpose+copy tail is emitted two ctx-tiles late so
the PE never stalls on the DVE normalization chain, wo's 8.4 MB load
is deferred behind the x strips, and the output is stored bf16 (host
upcasts) to halve the final DMA burst.
"""

from contextlib import ExitStack

import numpy as np
import ml_dtypes

import concourse.bass as bass
import concourse.bacc as bacc
import concourse.tile as tile
from concourse import mybir
from concourse.bass_utils import run_bass_kernel_spmd
from concourse.masks import make_identity
from concourse.tile_rust import add_dep_helper

BF16 = mybir.dt.bfloat16
F32 = mybir.dt.float32

B = 2
S = 2048
D = 2048
G = 4  # kv groups
HPG = 4  # heads per group
HD = 128  # head dim
QC = 512  # q-chunk (columns per S^T block)
NQC = S // QC  # 4
NKT = S // 128  # 16 k-tiles
NDC = D // 128  # 16 d_in chunks
SCALE = 1.0 / float(np.sqrt(HD))
N_CORES = 8
REPLICA_GROUPS = [[0, 1, 2, 3, 4, 5, 6, 7]]

CP = mybir.ActivationFunctionType.Copy
EXP = mybir.ActivationFunctionType.Exp


def _build_program(repeat: int = 1, sim: bool = False):
    nc = bacc.Bacc("TRN2", target_bir_lowering=False, debug=False)

    xq = nc.declare_dram_parameter("xq", [B, NQC, NDC, 128, QC], BF16, isOutput=False)
    wq = nc.declare_dram_parameter("wq", [NDC, 128, 2 * HD], BF16, isOutput=False)
    wk = nc.declare_dram_parameter("wk", [NDC, 128, HD], BF16, isOutput=False)
    wv = nc.declare_dram_parameter("wv", [NDC, 128, HD], BF16, isOutput=False)
    wo = nc.declare_dram_parameter("wo", [NDC, 128, D], BF16, isOutput=False)
    bo = nc.declare_dram_parameter("bo", [1, D], BF16, isOutput=False)
    msk = nc.declare_dram_parameter("msk", [128, 128], BF16, isOutput=False)
    out_ext = nc.declare_dram_parameter("out", [QC, D], BF16, isOutput=True)

    # AllToAll outputs: recv[j] block s = ctx^T of (g=s%4, h=2*(s//4)+j)
    # for this core's flat row chunk
    recv0 = nc.dram_tensor("recv0", [N_CORES, HD, QC], BF16)
    recv1 = nc.dram_tensor("recv1", [N_CORES, HD, QC], BF16)

    for _rep in range(repeat):
        _build_body(nc, _rep, xq, wq, wk, wv, wo, bo, msk, out_ext, recv0, recv1,
                    sim=sim)

    nc.compile()
    return nc


def _build_sim_program():
    """Single-core, collective-free variant of the body for TimelineSim."""
    nc = bacc.Bacc("TRN2", target_bir_lowering=False, debug=False)
    xq = nc.declare_dram_parameter("xq", [B, NQC, NDC, 128, QC], BF16, isOutput=False)
    wq = nc.declare_dram_parameter("wq", [NDC, 128, 2 * HD], BF16, isOutput=False)
    wk = nc.declare_dram_parameter("wk", [NDC, 128, HD], BF16, isOutput=False)
    wv = nc.declare_dram_parameter("wv", [NDC, 128, HD], BF16, isOutput=False)
    wo = nc.declare_dram_parameter("wo", [NDC, 128, D], BF16, isOutput=False)
    bo = nc.declare_dram_parameter("bo", [1, D], BF16, isOutput=False)
    msk = nc.declare_dram_parameter("msk", [128, 128], BF16, isOutput=False)
    out_ext = nc.declare_dram_parameter("out", [QC, D], BF16, isOutput=True)
    recv0 = nc.dram_tensor("recv0", [N_CORES, HD, QC], BF16)
    recv1 = nc.dram_tensor("recv1", [N_CORES, HD, QC], BF16)
    _build_body(nc, 0, xq, wq, wk, wv, wo, bo, msk, out_ext, recv0, recv1, sim=True)
    nc.compile()
    return nc


def _build_body(nc, _rep, xq, wq, wk, wv, wo, bo, msk, out_ext, recv0, recv1,
                sim=False):
    with tile.TileContext(nc) as tc, ExitStack() as es:
        singles = es.enter_context(tc.tile_pool(name="singles", bufs=1))
        wpool = es.enter_context(tc.tile_pool(name="w", bufs=1))
        qkpool = es.enter_context(tc.tile_pool(name="qk", bufs=1))
        apool = es.enter_context(tc.tile_pool(name="a", bufs=24))
        spool = es.enter_context(tc.tile_pool(name="sm", bufs=1))
        cpool = es.enter_context(tc.tile_pool(name="cs", bufs=6))
        opool = es.enter_context(tc.tile_pool(name="ob", bufs=2))
        pssc = es.enter_context(tc.tile_pool(name="pssc", bufs=4, space="PSUM"))
        pss = es.enter_context(tc.tile_pool(name="pss", bufs=2, space="PSUM"))
        pst = es.enter_context(tc.tile_pool(name="pst", bufs=2, space="PSUM"))
        dram = es.enter_context(tc.tile_pool(name="dram", bufs=1, space="DRAM"))
        dram2 = es.enter_context(tc.tile_pool(name="dram2", bufs=1, space="DRAM"))

        # --- constants (off the SP queue so x strip 0 starts immediately) ---
        ident = singles.tile([128, 128], BF16, tag="ident")
        make_identity(nc, ident)
        ones1 = singles.tile([1, 128], BF16, tag="ones1")
        nc.vector.memset(ones1, 1.0)
        bias_bc = singles.tile([128, D], BF16, tag="bias_bc")
        nc.scalar.dma_start(out=bias_bc, in_=bo[:, :].broadcast_to([128, D]))
        mask_sb = singles.tile([128, 128], BF16, tag="mask")
        nc.scalar.dma_start(out=mask_sb, in_=msk[:, :])
        # preload the exp activation table so phase 2's first exp is cheap
        warm = singles.tile([1, 4], F32, tag="warm")
        nc.scalar.activation(out=warm, in_=ones1[:, 0:4], func=EXP)

        # --- resident weights, loaded on the gpsimd queue (idle early).
        # wq is split so the first Q matmuls (dc 0..3) start sooner; wo
        # (8.4 MB, ~23 us of DMA) is deferred until the x strips are ahead
        # of the PE (it is only needed in phase 3) to avoid starving the
        # strip loads mid-phase-1.
        wqall = wpool.tile([128, NDC, 2 * HD], BF16, tag="wqall")
        nc.gpsimd.dma_start(
            out=wqall[:, 0:4, :], in_=wq[0:4].rearrange("a p q -> p a q")
        )
        nc.gpsimd.dma_start(
            out=wqall[:, 4:NDC, :], in_=wq[4:NDC].rearrange("a p q -> p a q")
        )
        wkall = wpool.tile([128, NDC, HD], BF16, tag="wkall")
        wvall = wpool.tile([128, NDC, HD], BF16, tag="wvall")
        woall = wpool.tile([128, NDC, D], BF16, tag="woall")
        kv_loaded = [False]
        wq_sb = [wqall[:, dc, :] for dc in range(NDC)]
        wk_sb = [wkall[:, dc, :] for dc in range(NDC)]
        wv_sb = [wvall[:, dc, :] for dc in range(NDC)]

        # --- persistent activations (per batch) ---
        qT = [
            [
                qkpool.tile([128, S], BF16, tag=f"qT{b}{j}", name=f"r{_rep}qT{b}{j}")
                for j in range(2)
            ]
            for b in range(B)
        ]
        kT = [
            qkpool.tile([128, S], BF16, tag=f"kT{b}", name=f"r{_rep}kT{b}") for b in range(B)
        ]
        vext = [
            [
                qkpool.tile([128, HD + 1], BF16, tag=f"v{b}_{i}", name=f"r{_rep}v{b}_{i}")
                for i in range(NKT)
            ]
            for b in range(B)
        ]

        ct_dram0 = dram.tile([N_CORES, HD, QC], BF16, tag="ct0", name=f"r{_rep}ctd0")
        ct_dram1 = dram2.tile([N_CORES, HD, QC], BF16, tag="ct1", name=f"r{_rep}ctd1")

        # Deferred-finish queue: the transpose (PE) + copy (DVE) tail of each
        # ctx tile is emitted LAG ctx-tiles late, so by the time the PE
        # in-order queue reaches a transpose its DVE-normalized input is
        # already done and the PE never stalls on the DVE chain.
        fin_q = []
        FIN_LAG = 2
        last_ct_ref = [None]

        def push_fin(f):
            fin_q.append(f)
            while len(fin_q) > FIN_LAG:
                fin_q.pop(0)()

        def flush_fins():
            while fin_q:
                fin_q.pop(0)()

        def attn_unit(b, j, qc):
            """Attention for (batch b, head-slot j, q-chunk qc); writes
            ctx^T [128, 512] to ct_dram[j] block 4*b+qc."""
            d0 = 4 * qc
            # 512-col PSUM bank slots; diagonal tiles trimmed to live cols.
            # Each slot gets its own 1-bank PSUM tile + exp, so a score
            # matmul only WAR-waits on the exp from 4 slots ago (not the
            # 1024-wide exp of the previous slot pair).
            slots = [[(kt, 0, QC)] for kt in range(d0)]
            slots.append([(d0, 0, QC)])
            slots.append([(d0 + 1, 128, 384), (d0 + 3, 384, 128)])
            slots.append([(d0 + 2, 256, 256)])  # partial slot last
            a_sl = {}

            def emit_slot(slot):
                ps = pssc.tile([128, QC], F32, tag="ssc")
                a = apool.tile([128, QC], BF16, tag="a")
                soff = 0
                for kt, qb, w in slot:
                    nc.tensor.matmul(
                        ps[:, soff : soff + w],
                        lhsT=kT[b][:, kt * 128 : (kt + 1) * 128],
                        rhs=qT[b][j][:, qc * QC + qb : (qc + 1) * QC],
                        start=True,
                        stop=True,
                    )
                    a_sl[kt] = (a, soff, qb)
                    soff += w
                nc.scalar.activation(
                    out=a[:, 0:soff], in_=ps[:, 0:soff], func=EXP, scale=SCALE
                )
                for kt, qb, w in slot:
                    if kt >= d0:  # diagonal: triangular mask
                        ao = a_sl[kt][1]
                        nc.vector.tensor_mul(
                            a[:, ao : ao + 128], a[:, ao : ao + 128], mask_sb
                        )

            ct = cpool.tile([128, QC], BF16, tag="ct", bufs=12)

            def emit_ctx(st):
                qt = qc * 4 + st
                cps = pss.tile([128, HD + 1], F32, tag="small")
                for kt in range(qt + 1):
                    a, ao, qb = a_sl[kt]
                    nc.tensor.matmul(
                        cps,
                        lhsT=a[:, ao + st * 128 - qb : ao + (st + 1) * 128 - qb],
                        rhs=vext[b][kt],
                        start=(kt == 0),
                        stop=(kt == qt),
                    )
                zr = cpool.tile([128, 1], F32, tag="zr", bufs=6)
                nc.vector.reciprocal(zr, cps[:, HD : HD + 1])
                cs = cpool.tile([128, HD], BF16, tag="cs", bufs=6)
                nc.vector.tensor_scalar_mul(cs, cps[:, 0:HD], zr)

                def fin(st=st, cs=cs):
                    tp = pst.tile([128, 128], BF16, tag="tp")
                    nc.tensor.transpose(tp, cs, ident)
                    nc.vector.tensor_copy(ct[:, st * 128 : (st + 1) * 128], tp)

                push_fin(fin)

            def emit_dma():
                if j == 0:
                    last_ct_ref[0] = nc.sync.dma_start(
                        out=ct_dram0[4 * b + qc], in_=ct
                    )
                else:
                    last_ct_ref[0] = nc.sync.dma_start(
                        out=ct_dram1[4 * b + qc], in_=ct
                    )

            # slots 0..d0+1 cover k-tiles d0, d0+1, d0+3 -> st 0,1,3 ready;
            # the final slot only adds d0+2 (needed by st 2,3)
            for slot in slots[:-1]:
                emit_slot(slot)
            emit_ctx(0)
            emit_ctx(1)
            emit_slot(slots[-1])
            emit_ctx(2)
            emit_ctx(3)
            push_fin(emit_dma)

        # ===== Phase 1 + interleaved j=0 attention (one chunk late, so each
        # unit's exp has a full projection chunk of slack before its ctx
        # matmuls reach the head of the in-order PE queue) =====
        with tc.tile_pool(name="x", bufs=2) as xpool:

            def proj_chunk(b, qc):
                xstrip = xpool.tile([128, NDC, QC], BF16, tag="xs")
                if b == 0 and qc == 0:
                    for dq in range(4):
                        nc.sync.dma_start(
                            out=xstrip[:, 4 * dq : 4 * (dq + 1), :],
                            in_=xq[b, qc, 4 * dq : 4 * (dq + 1)].rearrange(
                                "a p q -> p a q"
                            ),
                        )
                else:
                    strip_dmas.append(
                        nc.sync.dma_start(
                            out=xstrip, in_=xq[b, qc].rearrange("a p q -> p a q")
                        )
                    )
                xs = [xstrip[:, dc, :] for dc in range(NDC)]
                if not kv_loaded[0]:
                    kv_loaded[0] = True
                    nc.gpsimd.dma_start(
                        out=wkall, in_=wk.rearrange("a p q -> p a q")
                    )
                    nc.gpsimd.dma_start(
                        out=wvall, in_=wv.rearrange("a p q -> p a q")
                    )
                # Q^T per head-slot in its own 1-bank PSUM tile
                for j in range(2):
                    psq = pssc.tile([128, QC], F32, tag="ssc")
                    for dc in range(NDC):
                        nc.tensor.matmul(
                            psq,
                            lhsT=wq_sb[dc][:, j * HD : (j + 1) * HD],
                            rhs=xs[dc],
                            start=(dc == 0),
                            stop=(dc == NDC - 1),
                        )
                    nc.vector.tensor_copy(
                        qT[b][j][:, qc * QC : (qc + 1) * QC], psq
                    )
                # K^T: [dh, q 512]
                psk = pssc.tile([128, QC], F32, tag="ssc")
                for dc in range(NDC):
                    nc.tensor.matmul(
                        psk,
                        lhsT=wk_sb[dc],
                        rhs=xs[dc],
                        start=(dc == 0),
                        stop=(dc == NDC - 1),
                    )
                nc.vector.tensor_copy(kT[b][:, qc * QC : (qc + 1) * QC], psk)
                # V: [s-tile 128, dv 128] (natural orientation)
                for st in range(4):
                    kt = qc * 4 + st
                    ps = pss.tile([128, HD + 1], F32, tag="small")
                    for dc in range(NDC):
                        nc.tensor.matmul(
                            ps[:, 0:HD],
                            lhsT=xs[dc][:, st * 128 : (st + 1) * 128],
                            rhs=wv_sb[dc],
                            start=(dc == 0),
                            stop=(dc == NDC - 1),
                        )
                    nc.vector.tensor_copy(vext[b][kt][:, 0:HD], ps[:, 0:HD])
                    nc.vector.memset(vext[b][kt][:, HD : HD + 1], 1.0)

            strip_dmas = []
            chunks = [(b, qc) for b in range(B) for qc in range(NQC)]
            # j=1 units pulled into phase 1 (ACT has headroom there, while
            # the j-1-only phase 2 is exp/Activation-bound): after chunk i,
            # run the listed (b, qc) j=1 unit.
            j1_early = {2: (0, 0), 3: (0, 1), 5: (1, 0), 6: (1, 1), 7: (0, 2)}
            for i, (b, qc) in enumerate(chunks):
                proj_chunk(b, qc)
                if b == 0 and qc == 3:
                    # defer the big wo load until the x strips have a
                    # head start; it only matters in phase 3
                    wo_dma = nc.gpsimd.dma_start(
                        out=woall, in_=wo.rearrange("a p q -> p a q")
                    )
                    add_dep_helper(
                        wo_dma.ins, strip_dmas[-1].ins, reason="yield-to-strips"
                    )
                if i >= 1:
                    pb, pqc = chunks[i - 1]
                    attn_unit(pb, 0, pqc)
                if i in j1_early:
                    eb, eqc = j1_early[i]
                    attn_unit(eb, 1, eqc)
            attn_unit(chunks[-1][0], 0, chunks[-1][1])
            flush_fins()

        coll0 = None
        if not sim:
            coll0 = nc.gpsimd.collective_compute(
                "AllToAll",
                mybir.AluOpType.bypass,
                replica_groups=REPLICA_GROUPS,
                ins=[ct_dram0[:, :, :].opt()],
                outs=[recv0[:, :, :].opt()],
            )

        # ===== remaining j=1 attention (big units first; a medium one last
        # so the second collective's latency tail is short) =====
        for b, qc in ((0, 3), (1, 3), (1, 2)):
            attn_unit(b, 1, qc)
        flush_fins()
        last_ct = last_ct_ref[0]
        coll1 = None
        if not sim:
            coll1 = nc.gpsimd.collective_compute(
                "AllToAll",
                mybir.AluOpType.bypass,
                replica_groups=REPLICA_GROUPS,
                ins=[ct_dram1[:, :, :].opt()],
                outs=[recv1[:, :, :].opt()],
            )

        # ===== Phase 3: row-parallel output projection, two waves =====
        accp = es.enter_context(tc.tile_pool(name="acc", bufs=1))
        accs = [
            accp.tile([128, D], F32, tag=f"acc{st}", name=f"r{_rep}acc{st}")
            for st in range(4)
        ]
        cstrips = []
        src0 = ct_dram0 if sim else recv0
        src1 = ct_dram1 if sim else recv1
        cstrip0 = spool.tile([128, N_CORES, QC], BF16, tag="cstrip0")
        d = nc.sync.dma_start(
            out=cstrip0, in_=src0[:, :, :].rearrange("g p q -> p g q")
        )
        if not sim:
            add_dep_helper(d.ins, coll0.ins, reason="alltoall->read")
            # keep the scheduler from hoisting this read ahead of the j=1 ct
            # writes on the SP queue (it would chain them behind coll0)
            add_dep_helper(d.ins, last_ct.ins, reason="order-after-ct-writes")
        cstrips.append(cstrip0)
        cstrip1 = spool.tile([128, N_CORES, QC], BF16, tag="cstrip1")
        d = nc.sync.dma_start(
            out=cstrip1, in_=src1[:, :, :].rearrange("g p q -> p g q")
        )
        if not sim:
            add_dep_helper(d.ins, coll1.ins, reason="alltoall->read")
            add_dep_helper(d.ins, last_ct.ins, reason="order-after-ct-writes")
        cstrips.append(cstrip1)

        units = [(st, cc) for st in range(4) for cc in range(4)]
        unitsB = [(st, cc) for st in (0, 1, 2, 3) for cc in range(4)]

        def wave(j, final):
            ulist = unitsB if final else units
            for u, (st, cc) in enumerate(ulist):
                half = pssc.tile([128, QC], F32, tag="ssc")
                for s in range(N_CORES):
                    dch = 4 * (s % 4) + 2 * (s // 4) + j
                    nc.tensor.matmul(
                        half,
                        lhsT=cstrips[j][:, s, st * 128 : (st + 1) * 128],
                        rhs=woall[:, dch, cc * QC : (cc + 1) * QC],
                        start=(s == 0),
                        stop=(s == N_CORES - 1),
                    )
                if not final:
                    nc.vector.tensor_add(
                        accs[st][:, cc * QC : (cc + 1) * QC],
                        half,
                        bias_bc[:, cc * QC : (cc + 1) * QC],
                    )
                else:
                    osb = opool.tile([128, QC], BF16, tag="osb")
                    nc.vector.tensor_add(
                        osb, half, accs[st][:, cc * QC : (cc + 1) * QC]
                    )
                    nc.sync.dma_start(
                        out=out_ext[
                            st * 128 : (st + 1) * 128, cc * QC : (cc + 1) * QC
                        ],
                        in_=osb,
                    )

        wave(0, final=False)
        wave(1, final=True)


def _make_mask() -> np.ndarray:
    # mask[k, q] = 1.0 if q >= k (triangular causal for the [128,128]
    # diagonal sub-block of each diagonal k-tile)
    q = np.arange(128)[None, :]
    k = np.arange(128)[:, None]
    return (q >= k).astype(ml_dtypes.bfloat16)


def _make_in_maps(inputs) -> list[dict]:
    x = np.asarray(inputs["x"], dtype=np.float32)
    Wq = np.asarray(inputs["Wq"], dtype=np.float32)
    Wk = np.asarray(inputs["Wk"], dtype=np.float32)
    Wv = np.asarray(inputs["Wv"], dtype=np.float32)
    Wo = np.asarray(inputs["Wo"], dtype=np.float32)
    bo = np.asarray(inputs["bo"], dtype=np.float32)

    bf = ml_dtypes.bfloat16
    mask = _make_mask()

    # x^T tiled: [b, qc, dc, 128, 512], both batches shipped to every core
    xqs = []
    for b in range(B):
        xT = np.ascontiguousarray(x[b].T.astype(bf))  # [d, s]
        xqs.append(xT.reshape(NDC, 128, NQC, QC).transpose(2, 0, 1, 3))
    xq_all = np.ascontiguousarray(np.stack(xqs))

    wo_full = np.ascontiguousarray(Wo.astype(bf).reshape(NDC, 128, D))
    bo_full = np.ascontiguousarray(bo.astype(bf).reshape(1, D))

    in_maps = []
    for c in range(N_CORES):
        g, half = c % 4, c // 4
        q_lo = g * 512 + half * 2 * HD  # this core's two head-slots of group g
        in_maps.append(
            {
                "xq": xq_all,
                "wq": np.ascontiguousarray(
                    Wq[:, q_lo : q_lo + 2 * HD].astype(bf).reshape(NDC, 128, 2 * HD)
                ),
                "wk": np.ascontiguousarray(
                    Wk[:, g * HD : (g + 1) * HD].astype(bf).reshape(NDC, 128, HD)
                ),
                "wv": np.ascontiguousarray(
                    Wv[:, g * HD : (g + 1) * HD].astype(bf).reshape(NDC, 128, HD)
                ),
                "wo": wo_full,
                "bo": bo_full,
                "msk": mask,
            }
        )
    return in_maps


def _assemble(results) -> np.ndarray:
    out = np.empty((B, S, D), dtype=np.float32)
    for c in range(N_CORES):
        b, r = c // 4, c % 4
        out[b][r * QC : (r + 1) * QC, :] = results[c]["out"].astype(np.float32)
    return out


def kernel(**inputs) -> np.ndarray:
    in_maps = _make_in_maps(inputs)
    nc = _build_program()
    res = run_bass_kernel_spmd(nc, in_maps, list(range(N_CORES)))
    return _assemble(res.results)



# revision 35
# speedup vs baseline: 104.7281x; 8.2368x over previous
"""GroupedQueryAttention Trainium2 kernel (8 NeuronCores).

Sharding: core c -> (kv-group g = c%4, head-slot pair {2*(c//4), 2*(c//4)+1}).
Each core computes its group's two head-slots over BOTH batches (K/V are
computed per batch on every core), then two 8-rank AllToAlls (one per
head-slot j) redistribute ctx^T so core c ends up with ctx^T of ALL 16
heads for its own flat row chunk c (batch c//4, rows 512*(c%4)..+512).
The output projection is then row-parallel with the full Wo resident —
no further collectives.

Every AllToAll block is useful: src s = (g=s%4, half=s//4) contributes
head (g, 2*half+j) of batch d//4, rows 512*(d%4).. to dest d, and the
d_model chunk index of block (s, j) is the compile-time constant
4*(s%4) + 2*(s//4) + j — pure SPMD, no rank-dependent indexing.

Pipelining: the attention phase is Activation-engine-bound (each
512-col exp costs ~610 ns vs ~530 ns of PE work per score slot), so
attention units are spread into the projection phase, which has ACT
headroom: every j=0 unit runs one chunk behind its projections, and
five of the eight j=1 units are pulled forward too (j1_early).  Only
three j=1 units remain after the first AllToAll, which therefore
issues early and hides; the second AllToAll is covered by wave 0 of
the output projection (j=0 terms + broadcast bias accumulate into an
SBUF fp32 accumulator), with the j=1 terms and the bf16 add/store in
wave 1 after it lands.

Layout tricks: scores are computed transposed (S^T[k, q]) so A^T =
exp(S^T) is directly the lhsT of the ctx matmul.  The softmax
denominator comes free as a 129th "ones" column appended to V; ctx
rows are normalized by a per-partition reciprocal scale.  Each score
slot owns a single 1-bank [128,512] PSUM tile with its own exp, so a
score matmul only WAR-waits on the exp four slots back.  All
PSUM->SBUF projection copies run on the DVE (not ACT), the bias is
materialized once by a stride-0 broadcast DMA (no per-tile bias
matmuls), the ctx transpose+copy tail is emitted two ctx-tiles late so
the PE never stalls on the DVE normalization chain, wo's 8.4 MB load
is deferred behind the x strips, and the output is stored bf16 (host
upcasts) to halve the final DMA burst.
"""

from contextlib import ExitStack

import numpy as np
import ml_dtypes

import concourse.bass as bass
import concourse.bacc as bacc
import concourse.tile as tile
from concourse import mybir
from concourse.bass_utils import run_bass_kernel_spmd
from concourse.masks import make_identity
from concourse.tile_rust import add_dep_helper

BF16 = mybir.dt.bfloat16
F32 = mybir.dt.float32

B = 2
S = 2048
D = 2048
G = 4  # kv groups
HPG = 4  # heads per group
HD = 128  # head dim
QC = 512  # q-chunk (columns per S^T block)
NQC = S // QC  # 4
NKT = S // 128  # 16 k-tiles
NDC = D // 128  # 16 d_in chunks
SCALE = 1.0 / float(np.sqrt(HD))
N_CORES = 8
REPLICA_GROUPS = [[0, 1, 2, 3, 4, 5, 6, 7]]

CP = mybir.ActivationFunctionType.Copy
EXP = mybir.ActivationFunctionType.Exp


def _build_program(repeat: int = 1, sim: bool = False):
    nc = bacc.Bacc("TRN2", target_bir_lowering=False, debug=False)

    xq = nc.declare_dram_parameter("xq", [B, NQC, NDC, 128, QC], BF16, isOutput=False)
    wq = nc.declare_dram_parameter("wq", [NDC, 128, 2 * HD], BF16, isOutput=False)
    wk = nc.declare_dram_parameter("wk", [NDC, 128, HD], BF16, isOutput=False)
    wv = nc.declare_dram_parameter("wv", [NDC, 128, HD], BF16, isOutput=False)
    wo = nc.declare_dram_parameter("wo", [NDC, 128, D], BF16, isOutput=False)
    bo = nc.declare_dram_parameter("bo", [1, D], BF16, isOutput=False)
    msk = nc.declare_dram_parameter("msk", [128, 128], BF16, isOutput=False)
    out_ext = nc.declare_dram_parameter("out", [QC, D], BF16, isOutput=True)

    # AllToAll outputs: recv[j] block s = ctx^T of (g=s%4, h=2*(s//4)+j)
    # for this core's flat row chunk
    recv0 = nc.dram_tensor("recv0", [N_CORES, HD, QC], BF16)
    recv1 = nc.dram_tensor("recv1", [N_CORES, HD, QC], BF16)

    for _rep in range(repeat):
        _build_body(nc, _rep, xq, wq, wk, wv, wo, bo, msk, out_ext, recv0, recv1,
                    sim=sim)

    nc.compile()
    return nc


def _build_sim_program():
    """Single-core, collective-free variant of the body for TimelineSim."""
    nc = bacc.Bacc("TRN2", target_bir_lowering=False, debug=False)
    xq = nc.declare_dram_parameter("xq", [B, NQC, NDC, 128, QC], BF16, isOutput=False)
    wq = nc.declare_dram_parameter("wq", [NDC, 128, 2 * HD], BF16, isOutput=False)
    wk = nc.declare_dram_parameter("wk", [NDC, 128, HD], BF16, isOutput=False)
    wv = nc.declare_dram_parameter("wv", [NDC, 128, HD], BF16, isOutput=False)
    wo = nc.declare_dram_parameter("wo", [NDC, 128, D], BF16, isOutput=False)
    bo = nc.declare_dram_parameter("bo", [1, D], BF16, isOutput=False)
    msk = nc.declare_dram_parameter("msk", [128, 128], BF16, isOutput=False)
    out_ext = nc.declare_dram_parameter("out", [QC, D], BF16, isOutput=True)
    recv0 = nc.dram_tensor("recv0", [N_CORES, HD, QC], BF16)
    recv1 = nc.dram_tensor("recv1", [N_CORES, HD, QC], BF16)
    _build_body(nc, 0, xq, wq, wk, wv, wo, bo, msk, out_ext, recv0, recv1, sim=True)
    nc.compile()
    return nc


def _build_body(nc, _rep, xq, wq, wk, wv, wo, bo, msk, out_ext, recv0, recv1,
                sim=False):
    with tile.TileContext(nc) as tc, ExitStack() as es:
        singles = es.enter_context(tc.tile_pool(name="singles", bufs=1))
        wpool = es.enter_context(tc.tile_pool(name="w", bufs=1))
        qkpool = es.enter_context(tc.tile_pool(name="qk", bufs=1))
        apool = es.enter_context(tc.tile_pool(name="a", bufs=24))
        spool = es.enter_context(tc.tile_pool(name="sm", bufs=1))
        cpool = es.enter_context(tc.tile_pool(name="cs", bufs=6))
        opool = es.enter_context(tc.tile_pool(name="ob", bufs=2))
        pssc = es.enter_context(tc.tile_pool(name="pssc", bufs=4, space="PSUM"))
        pss = es.enter_context(tc.tile_pool(name="pss", bufs=2, space="PSUM"))
        pst = es.enter_context(tc.tile_pool(name="pst", bufs=2, space="PSUM"))
        dram = es.enter_context(tc.tile_pool(name="dram", bufs=1, space="DRAM"))
        dram2 = es.enter_context(tc.tile_pool(name="dram2", bufs=1, space="DRAM"))

        # --- constants (off the SP queue so x strip 0 starts immediately) ---
        ident = singles.tile([128, 128], BF16, tag="ident")
        make_identity(nc, ident)
        ones1 = singles.tile([1, 128], BF16, tag="ones1")
        nc.vector.memset(ones1, 1.0)
        bias_bc = singles.tile([128, D], BF16, tag="bias_bc")
        nc.scalar.dma_start(out=bias_bc, in_=bo[:, :].broadcast_to([128, D]))
        mask_sb = singles.tile([128, 128], BF16, tag="mask")
        nc.scalar.dma_start(out=mask_sb, in_=msk[:, :])
        # preload the exp activation table so phase 2's first exp is cheap
        warm = singles.tile([1, 4], F32, tag="warm")
        nc.scalar.activation(out=warm, in_=ones1[:, 0:4], func=EXP)

        # --- resident weights, loaded on the gpsimd queue (idle early).
        # wq is split so the first Q matmuls (dc 0..3) start sooner; wo
        # (8.4 MB, ~23 us of DMA) is deferred until the x strips are ahead
        # of the PE (it is only needed in phase 3) to avoid starving the
        # strip loads mid-phase-1.
        wqall = wpool.tile([128, NDC, 2 * HD], BF16, tag="wqall")
        nc.gpsimd.dma_start(
            out=wqall[:, 0:4, :], in_=wq[0:4].rearrange("a p q -> p a q")
        )
        nc.gpsimd.dma_start(
            out=wqall[:, 4:NDC, :], in_=wq[4:NDC].rearrange("a p q -> p a q")
        )
        wkall = wpool.tile([128, NDC, HD], BF16, tag="wkall")
        wvall = wpool.tile([128, NDC, HD], BF16, tag="wvall")
        woall = wpool.tile([128, NDC, D], BF16, tag="woall")
        kv_loaded = [False]
        wq_sb = [wqall[:, dc, :] for dc in range(NDC)]
        wk_sb = [wkall[:, dc, :] for dc in range(NDC)]
        wv_sb = [wvall[:, dc, :] for dc in range(NDC)]

        # --- persistent activations (per batch) ---
        qT = [
            [
                qkpool.tile([128, S], BF16, tag=f"qT{b}{j}", name=f"r{_rep}qT{b}{j}")
                for j in range(2)
            ]
            for b in range(B)
        ]
        kT = [
            qkpool.tile([128, S], BF16, tag=f"kT{b}", name=f"r{_rep}kT{b}") for b in range(B)
        ]
        vext = [
            [
                qkpool.tile([128, HD + 1], BF16, tag=f"v{b}_{i}", name=f"r{_rep}v{b}_{i}")
                for i in range(NKT)
            ]
            for b in range(B)
        ]

        ct_dram0 = dram.tile([N_CORES, HD, QC], BF16, tag="ct0", name=f"r{_rep}ctd0")
        ct_dram1 = dram2.tile([N_CORES, HD, QC], BF16, tag="ct1", name=f"r{_rep}ctd1")

        # Deferred-finish queue: the transpose (PE) + copy (DVE) tail of each
        # ctx tile is emitted LAG ctx-tiles late, so by the time the PE
        # in-order queue reaches a transpose its DVE-normalized input is
        # already done and the PE never stalls on the DVE chain.
        fin_q = []
        FIN_LAG = 2
        last_ct_ref = [None]

        def push_fin(f):
            fin_q.append(f)
            while len(fin_q) > FIN_LAG:
                fin_q.pop(0)()

        def flush_fins():
            while fin_q:
                fin_q.pop(0)()

        def attn_unit_gen(b, j, qc):
            """Attention for (batch b, head-slot j, q-chunk qc); writes
            ctx^T [128, 512] to ct_dram[j] block 4*b+qc.  A generator
            yielding after each slot/ctx segment so the phase-1 driver can
            interleave projection matmuls between attention segments (the
            exp chain makes attention ACT-bound; proj work fills the PE
            while ACT catches up)."""
            d0 = 4 * qc
            # 512-col PSUM bank slots; diagonal tiles trimmed to live cols.
            # Each slot gets its own 1-bank PSUM tile + exp, so a score
            # matmul only WAR-waits on the exp from 4 slots ago (not the
            # 1024-wide exp of the previous slot pair).
            slots = [[(kt, 0, QC)] for kt in range(d0)]
            slots.append([(d0, 0, QC)])
            slots.append([(d0 + 1, 128, 384), (d0 + 3, 384, 128)])
            slots.append([(d0 + 2, 256, 256)])  # partial slot last
            a_sl = {}

            def emit_slot(slot):
                ps = pssc.tile([128, QC], F32, tag="ssc")
                a = apool.tile([128, QC], BF16, tag="a")
                soff = 0
                for kt, qb, w in slot:
                    nc.tensor.matmul(
                        ps[:, soff : soff + w],
                        lhsT=kT[b][:, kt * 128 : (kt + 1) * 128],
                        rhs=qT[b][j][:, qc * QC + qb : (qc + 1) * QC],
                        start=True,
                        stop=True,
                    )
                    a_sl[kt] = (a, soff, qb)
                    soff += w
                nc.scalar.activation(
                    out=a[:, 0:soff], in_=ps[:, 0:soff], func=EXP, scale=SCALE
                )
                for kt, qb, w in slot:
                    if kt >= d0:  # diagonal: triangular mask
                        ao = a_sl[kt][1]
                        nc.vector.tensor_mul(
                            a[:, ao : ao + 128], a[:, ao : ao + 128], mask_sb
                        )

            ct = cpool.tile([128, QC], BF16, tag="ct", bufs=12)

            def emit_ctx(st):
                qt = qc * 4 + st
                cps = pss.tile([128, HD + 1], F32, tag="small")
                for kt in range(qt + 1):
                    a, ao, qb = a_sl[kt]
                    nc.tensor.matmul(
                        cps,
                        lhsT=a[:, ao + st * 128 - qb : ao + (st + 1) * 128 - qb],
                        rhs=vext[b][kt],
                        start=(kt == 0),
                        stop=(kt == qt),
                    )
                zr = cpool.tile([128, 1], F32, tag="zr", bufs=6)
                nc.vector.reciprocal(zr, cps[:, HD : HD + 1])
                cs = cpool.tile([128, HD], BF16, tag="cs", bufs=6)
                nc.vector.tensor_scalar_mul(cs, cps[:, 0:HD], zr)

                def fin(st=st, cs=cs):
                    tp = pst.tile([128, 128], BF16, tag="tp")
                    nc.tensor.transpose(tp, cs, ident)
                    nc.vector.tensor_copy(ct[:, st * 128 : (st + 1) * 128], tp)

                push_fin(fin)

            def emit_dma():
                if j == 0:
                    last_ct_ref[0] = nc.sync.dma_start(
                        out=ct_dram0[4 * b + qc], in_=ct
                    )
                else:
                    last_ct_ref[0] = nc.sync.dma_start(
                        out=ct_dram1[4 * b + qc], in_=ct
                    )

            # slots 0..d0+1 cover k-tiles d0, d0+1, d0+3 -> st 0,1,3 ready;
            # the final slot only adds d0+2 (needed by st 2,3)
            for slot in slots[:-1]:
                emit_slot(slot)
                yield
            emit_ctx(0)
            yield
            emit_ctx(1)
            yield
            emit_slot(slots[-1])
            yield
            emit_ctx(2)
            yield
            emit_ctx(3)
            push_fin(emit_dma)

        def attn_unit(b, j, qc):
            for _ in attn_unit_gen(b, j, qc):
                pass

        # ===== Phase 1 + interleaved j=0 attention (one chunk late, so each
        # unit's exp has a full projection chunk of slack before its ctx
        # matmuls reach the head of the in-order PE queue) =====
        with tc.tile_pool(name="x", bufs=2) as xpool:

            def proj_chunk_gen(b, qc):
                xstrip = xpool.tile([128, NDC, QC], BF16, tag="xs")
                if b == 0 and qc == 0:
                    for dq in range(4):
                        nc.sync.dma_start(
                            out=xstrip[:, 4 * dq : 4 * (dq + 1), :],
                            in_=xq[b, qc, 4 * dq : 4 * (dq + 1)].rearrange(
                                "a p q -> p a q"
                            ),
                        )
                else:
                    strip_dmas.append(
                        nc.sync.dma_start(
                            out=xstrip, in_=xq[b, qc].rearrange("a p q -> p a q")
                        )
                    )
                xs = [xstrip[:, dc, :] for dc in range(NDC)]
                if not kv_loaded[0]:
                    kv_loaded[0] = True
                    nc.gpsimd.dma_start(
                        out=wkall, in_=wk.rearrange("a p q -> p a q")
                    )
                    nc.gpsimd.dma_start(
                        out=wvall, in_=wv.rearrange("a p q -> p a q")
                    )
                # Q^T per head-slot in its own 1-bank PSUM tile
                for j in range(2):
                    psq = pssc.tile([128, QC], F32, tag="ssc")
                    for dc in range(NDC):
                        nc.tensor.matmul(
                            psq,
                            lhsT=wq_sb[dc][:, j * HD : (j + 1) * HD],
                            rhs=xs[dc],
                            start=(dc == 0),
                            stop=(dc == NDC - 1),
                        )
                    nc.vector.tensor_copy(
                        qT[b][j][:, qc * QC : (qc + 1) * QC], psq
                    )
                    yield
                # K^T: [dh, q 512]
                psk = pssc.tile([128, QC], F32, tag="ssc")
                for dc in range(NDC):
                    nc.tensor.matmul(
                        psk,
                        lhsT=wk_sb[dc],
                        rhs=xs[dc],
                        start=(dc == 0),
                        stop=(dc == NDC - 1),
                    )
                nc.vector.tensor_copy(kT[b][:, qc * QC : (qc + 1) * QC], psk)
                yield
                # V: [s-tile 128, dv 128] (natural orientation)
                for st in range(4):
                    kt = qc * 4 + st
                    ps = pss.tile([128, HD + 1], F32, tag="small")
                    for dc in range(NDC):
                        nc.tensor.matmul(
                            ps[:, 0:HD],
                            lhsT=xs[dc][:, st * 128 : (st + 1) * 128],
                            rhs=wv_sb[dc],
                            start=(dc == 0),
                            stop=(dc == NDC - 1),
                        )
                    nc.vector.tensor_copy(vext[b][kt][:, 0:HD], ps[:, 0:HD])
                    nc.vector.memset(vext[b][kt][:, HD : HD + 1], 1.0)
                    yield

            strip_dmas = []
            chunks = [(b, qc) for b in range(B) for qc in range(NQC)]
            # j=1 units pulled into phase 1 (ACT has headroom there, while
            # a j=1-only phase 2 is exp/Activation-bound): after chunk i,
            # the listed (b, qc) j=1 unit becomes available.
            j1_early = {2: (0, 0), 3: (0, 1), 5: (1, 0), 6: (1, 1), 7: (0, 2)}
            # Fine-grained interleave: drive proj and attention generators
            # round-robin (1 proj segment : ~3 unit segments -- there are
            # ~3x more unit segments than proj segments in phase 1), so the
            # PE alternates between proj matmuls and attention segments and
            # never outruns the ACT exp pipeline.
            unit_q = []

            def drive_units(k):
                for _ in range(k):
                    if not unit_q:
                        return
                    try:
                        next(unit_q[0])
                    except StopIteration:
                        unit_q.pop(0)

            for i, (b, qc) in enumerate(chunks):
                pg = proj_chunk_gen(b, qc)
                for seg in pg:
                    drive_units(3)
                if 3 <= i <= 6:
                    # wo (8.4 MB) is only needed in phase 3; load it in four
                    # 2.1 MB pieces, each gated on a mid-phase strip DMA, so
                    # no single wo transfer monopolizes the DMA engines while
                    # a strip the PE is waiting for is in flight
                    p = i - 3
                    wo_dma = nc.gpsimd.dma_start(
                        out=woall[:, 4 * p : 4 * (p + 1), :],
                        in_=wo[4 * p : 4 * (p + 1)].rearrange("a p q -> p a q"),
                    )
                    add_dep_helper(
                        wo_dma.ins, strip_dmas[-1].ins, reason="yield-to-strips"
                    )
                if i >= 1:
                    pb, pqc = chunks[i - 1]
                    unit_q.append(attn_unit_gen(pb, 0, pqc))
                if i in j1_early:
                    eb, eqc = j1_early[i]
                    unit_q.append(attn_unit_gen(eb, 1, eqc))
            unit_q.append(attn_unit_gen(chunks[-1][0], 0, chunks[-1][1]))
            while unit_q:
                drive_units(1)
            flush_fins()

        coll0 = None
        if not sim:
            coll0 = nc.gpsimd.collective_compute(
                "AllToAll",
                mybir.AluOpType.bypass,
                replica_groups=REPLICA_GROUPS,
                ins=[ct_dram0[:, :, :].opt()],
                outs=[recv0[:, :, :].opt()],
            )

        # ===== remaining j=1 attention (big units first; a medium one last
        # so the second collective's latency tail is short) =====
        for b, qc in ((0, 3), (1, 3), (1, 2)):
            attn_unit(b, 1, qc)
        flush_fins()
        last_ct = last_ct_ref[0]
        coll1 = None
        if not sim:
            coll1 = nc.gpsimd.collective_compute(
                "AllToAll",
                mybir.AluOpType.bypass,
                replica_groups=REPLICA_GROUPS,
                ins=[ct_dram1[:, :, :].opt()],
                outs=[recv1[:, :, :].opt()],
            )

        # ===== Phase 3: row-parallel output projection, two waves =====
        accp = es.enter_context(tc.tile_pool(name="acc", bufs=1))
        accs = [
            accp.tile([128, D], F32, tag=f"acc{st}", name=f"r{_rep}acc{st}")
            for st in range(4)
        ]
        cstrips = []
        src0 = ct_dram0 if sim else recv0
        src1 = ct_dram1 if sim else recv1
        cstrip0 = spool.tile([128, N_CORES, QC], BF16, tag="cstrip0")
        d = nc.sync.dma_start(
            out=cstrip0, in_=src0[:, :, :].rearrange("g p q -> p g q")
        )
        if not sim:
            add_dep_helper(d.ins, coll0.ins, reason="alltoall->read")
            # keep the scheduler from hoisting this read ahead of the j=1 ct
            # writes on the SP queue (it would chain them behind coll0)
            add_dep_helper(d.ins, last_ct.ins, reason="order-after-ct-writes")
        cstrips.append(cstrip0)
        cstrip1 = spool.tile([128, N_CORES, QC], BF16, tag="cstrip1")
        d = nc.sync.dma_start(
            out=cstrip1, in_=src1[:, :, :].rearrange("g p q -> p g q")
        )
        if not sim:
            add_dep_helper(d.ins, coll1.ins, reason="alltoall->read")
            add_dep_helper(d.ins, last_ct.ins, reason="order-after-ct-writes")
        cstrips.append(cstrip1)

        units = [(st, cc) for st in range(4) for cc in range(4)]
        unitsB = [(st, cc) for st in (0, 1, 2, 3) for cc in range(4)]

        def wave(j, final):
            ulist = unitsB if final else units
            for u, (st, cc) in enumerate(ulist):
                half = pssc.tile([128, QC], F32, tag="ssc")
                for s in range(N_CORES):
                    dch = 4 * (s % 4) + 2 * (s // 4) + j
                    nc.tensor.matmul(
                        half,
                        lhsT=cstrips[j][:, s, st * 128 : (st + 1) * 128],
                        rhs=woall[:, dch, cc * QC : (cc + 1) * QC],
                        start=(s == 0),
                        stop=(s == N_CORES - 1),
                    )
                if not final:
                    nc.vector.tensor_add(
                        accs[st][:, cc * QC : (cc + 1) * QC],
                        half,
                        bias_bc[:, cc * QC : (cc + 1) * QC],
                    )
                else:
                    osb = opool.tile([128, QC], BF16, tag="osb")
                    nc.vector.tensor_add(
                        osb, half, accs[st][:, cc * QC : (cc + 1) * QC]
                    )
                    nc.sync.dma_start(
                        out=out_ext[
                            st * 128 : (st + 1) * 128, cc * QC : (cc + 1) * QC
                        ],
                        in_=osb,
                    )

        wave(0, final=False)
        wave(1, final=True)


def _make_mask() -> np.ndarray:
    # mask[k, q] = 1.0 if q >= k (triangular causal for the [128,128]
    # diagonal sub-block of each diagonal k-tile)
    q = np.arange(128)[None, :]
    k = np.arange(128)[:, None]
    return (q >= k).astype(ml_dtypes.bfloat16)


def _make_in_maps(inputs) -> list[dict]:
    x = np.asarray(inputs["x"], dtype=np.float32)
    Wq = np.asarray(inputs["Wq"], dtype=np.float32)
    Wk = np.asarray(inputs["Wk"], dtype=np.float32)
    Wv = np.asarray(inputs["Wv"], dtype=np.float32)
    Wo = np.asarray(inputs["Wo"], dtype=np.float32)
    bo = np.asarray(inputs["bo"], dtype=np.float32)

    bf = ml_dtypes.bfloat16
    mask = _make_mask()

    # x^T tiled: [b, qc, dc, 128, 512], both batches shipped to every core
    xqs = []
    for b in range(B):
        xT = np.ascontiguousarray(x[b].T.astype(bf))  # [d, s]
        xqs.append(xT.reshape(NDC, 128, NQC, QC).transpose(2, 0, 1, 3))
    xq_all = np.ascontiguousarray(np.stack(xqs))

    wo_full = np.ascontiguousarray(Wo.astype(bf).reshape(NDC, 128, D))
    bo_full = np.ascontiguousarray(bo.astype(bf).reshape(1, D))

    in_maps = []
    for c in range(N_CORES):
        g, half = c % 4, c // 4
        q_lo = g * 512 + half * 2 * HD  # this core's two head-slots of group g
        in_maps.append(
            {
                "xq": xq_all,
                "wq": np.ascontiguousarray(
                    Wq[:, q_lo : q_lo + 2 * HD].astype(bf).reshape(NDC, 128, 2 * HD)
                ),
                "wk": np.ascontiguousarray(
                    Wk[:, g * HD : (g + 1) * HD].astype(bf).reshape(NDC, 128, HD)
                ),
                "wv": np.ascontiguousarray(
                    Wv[:, g * HD : (g + 1) * HD].astype(bf).reshape(NDC, 128, HD)
                ),
                "wo": wo_full,
                "bo": bo_full,
                "msk": mask,
            }
        )
    return in_maps


def _assemble(results) -> np.ndarray:
    out = np.empty((B, S, D), dtype=np.float32)
    for c in range(N_CORES):
        b, r = c // 4, c % 4
        out[b][r * QC : (r + 1) * QC, :] = results[c]["out"].astype(np.float32)
    return out


def kernel(**inputs) -> np.ndarray:
    in_maps = _make_in_maps(inputs)
    nc = _build_program()
    res = run_bass_kernel_spmd(nc, in_maps, list(range(N_CORES)))
    return _assemble(res.results)

